# revision 20
# baseline (speedup 1.0000x reference)
"""MultiHeadAttention (B=4,T=2048,D=2048,NQ=16,NK=8,H=128) on 8 trn2 cores.

Sharding: core c -> batch b=c//2, half=c%2. Each core computes the partial
output for batch b restricted to q-heads [half*8, half*8+8) (kv-heads
[half*4, half*4+4)); host sums the two partials per batch (o_proj
contraction over heads is split across the core pair).

v2: bf16 matmul/vector datapath (PSUM accumulation stays f32), fused
q/k/v projection pass with SBUF-resident q (no DRAM spill), DMA-engine
transposes for V, batched RMSNorm row statistics, f32r broadcast
matmuls, causal-restricted score/exp/PV tiles, software-pipelined
exp/PV attention loop with o_proj fused per 512-row query chunk.
"""
import numpy as np
import concourse.bass as bass
import concourse.tile as tile
from concourse import bacc, mybir
from concourse import bass_utils

B, T, D = 4, 2048, 2048
NQ, NK, H = 16, 8, 128
NH, NKV = 8, 4          # per-core q heads / kv heads
THETA = 10000.0
EPS = 1e-6
TCH = 512               # chunk of T for projections / attention q blocks
NCH = T // TCH
NDK = D // 128
NQC = T // TCH

f32 = mybir.dt.float32
f32r = mybir.dt.float32r
bf16 = mybir.dt.bfloat16
npbf16 = mybir.dt.np(bf16)
AF = mybir.ActivationFunctionType
MUL = mybir.AluOpType.mult

TRACE = False
LAST_EXEC_NS = None
_CACHE = {}


def _install_hook():
    import contextlib, ctypes, sys, types
    if "antenv.axon_hooks" in sys.modules:
        return
    lib = ctypes.CDLL("/opt/axon/libaxon_pjrt.so")
    lib.axon_start_nrt_profile.argtypes = [ctypes.POINTER(ctypes.c_int64), ctypes.c_size_t]
    lib.axon_start_nrt_profile.restype = ctypes.c_int64
    lib.axon_stop_nrt_profile.argtypes = [ctypes.c_char_p]
    lib.axon_stop_nrt_profile.restype = ctypes.c_int64

    @contextlib.contextmanager
    def _hook(output_dir, device_ids):
        import jax
        jax.devices()
        ids = (ctypes.c_int64 * len(device_ids))(*device_ids) if device_ids else None
        rc = lib.axon_start_nrt_profile(ids, len(device_ids) if device_ids else 0)
        if rc != 0:
            raise RuntimeError(f"axon_start_nrt_profile rc={rc}")
        try:
            yield
        finally:
            n = lib.axon_stop_nrt_profile(str(output_dir).encode())
            if n < 0:
                raise RuntimeError(f"axon_stop_nrt_profile rc={n}")

    mod = types.ModuleType("antenv.axon_hooks")
    mod.get_axon_ntff_profile_hook = lambda: _hook
    mod.set_axon_ntff_profile_hook = lambda h: None
    sys.modules["antenv.axon_hooks"] = mod
    bass_utils.upload_artifacts = lambda tmpdir: "local://" + str(tmpdir)


def _build():
    nc = bacc.Bacc("TRN2", target_bir_lowering=False, debug=False, num_devices=8)
    xt_ap = nc.dram_tensor("xt", [D, T], bf16, kind="ExternalInput").ap()
    wq_ap = nc.dram_tensor("wq", [128, NH * NDK * 128], bf16, kind="ExternalInput").ap()
    wk_ap = nc.dram_tensor("wk", [128, NKV * NDK * 128], bf16, kind="ExternalInput").ap()
    wv_ap = nc.dram_tensor("wv", [128, NKV * NDK * 128], bf16, kind="ExternalInput").ap()
    wo_ap = nc.dram_tensor("wo", [128, NH * D], bf16, kind="ExternalInput").ap()
    cs_ap = nc.dram_tensor("cs", [128, T], bf16, kind="ExternalInput").ap()
    sn_ap = nc.dram_tensor("sn", [128, T], bf16, kind="ExternalInput").ap()
    qsc_ap = nc.dram_tensor("qsc", [128, 1], f32, kind="ExternalInput").ap()
    ksc_ap = nc.dram_tensor("ksc", [128, 1], f32, kind="ExternalInput").ap()
    cm_ap = nc.dram_tensor("cm", [128, 128], bf16, kind="ExternalInput").ap()
    out_ap = nc.dram_tensor("out", [T, D], bf16, kind="ExternalOutput").ap()

    with tile.TileContext(nc) as tc:
        with tc.tile_pool(name="mp", bufs=1) as mp, \
             tc.tile_pool(name="pp", bufs=1, space="PSUM") as pp:
            # ---- persistent tiles ----
            cs_t = mp.tile([128, T], bf16)
            nc.sync.dma_start(cs_t[:], cs_ap[:])
            sn_t = mp.tile([128, T], bf16)
            nc.sync.dma_start(sn_t[:], sn_ap[:])
            qsc_t = mp.tile([128, 1], f32)
            nc.sync.dma_start(qsc_t[:], qsc_ap[:])
            ksc_t = mp.tile([128, 1], f32)
            nc.sync.dma_start(ksc_t[:], ksc_ap[:])
            cm_t = mp.tile([128, 128], bf16)
            nc.sync.dma_start(cm_t[:], cm_ap[:])
            eps_t = mp.tile([1, 1], f32)
            nc.vector.memset(eps_t[:], EPS)
            ones_col_b = mp.tile([128, 1], bf16)
            nc.vector.memset(ones_col_b[:], 1.0)
            ones_row_b = mp.tile([1, 128], bf16)
            nc.vector.memset(ones_row_b[:], 1.0)

            qT = mp.tile([128, NH * T], bf16)     # 32KB/part
            kT = mp.tile([128, NKV * T], bf16)    # 16KB
            vT = mp.tile([128, NKV * T], bf16)    # 16KB

            # PSUM bank for softmax denominator rows (attention phase).
            rows = pp.tile([128, 512], f32, tag="rows")

            def drain_group(pool, accs, sc_t, t0, dsts):
                """accs: psum [128,512] f32 -> RMSNorm*(gain) + RoPE -> dsts bf16."""
                n = len(accs)
                for i in range(n):
                    sq = pool.tile([128, TCH], bf16, tag="sq", bufs=2, name=f"sq{i}")
                    nc.scalar.activation(sq[:], accs[i], AF.Square)
                    row = pp.tile([1, 512], f32, tag=f"a{3 + i % 2}", name=f"row{i}")
                    nc.tensor.matmul(row[:], ones_col_b[:], sq[:], start=True, stop=True)
                    rinv = pool.tile([1, TCH], f32, tag="rinv", bufs=2, name=f"rinv{i}")
                    nc.vector.reciprocal_approx_fast(rinv[:], row[:])
                    rstd = pool.tile([1, TCH], bf16, tag="rstd", bufs=2, name=f"rstd{i}")
                    nc.scalar.activation(rstd[:], rinv[:], AF.Sqrt)
                    bc = pp.tile([128, 512], f32, tag=f"a{3 + (i + 1) % 2}", name=f"bc{i}")
                    nc.tensor.matmul(bc[:], ones_row_b[:], rstd[:],
                                     start=True, stop=True)
                    # qn0 = acc * gain (per-partition); rstd applied after rope
                    # (valid: rstd is per-column, invariant under the half-swap)
                    qn = pool.tile([128, TCH], bf16, tag="qn", bufs=2, name=f"qn{i}")
                    nc.scalar.activation(qn[:], accs[i], AF.Copy, scale=sc_t[:])
                    qsw = pool.tile([128, TCH], bf16, tag="qsw", bufs=2, name=f"qsw{i}")
                    nc.sync.dma_start(qsw[0:64, :], qn[64:128, :])
                    nc.sync.dma_start(qsw[64:128, :], qn[0:64, :])
                    ta = pool.tile([128, TCH], bf16, tag="ta", bufs=2, name=f"ta{i}")
                    nc.vector.tensor_mul(ta[:], qn[:], cs_t[:, t0:t0 + TCH])
                    tb = pool.tile([128, TCH], bf16, tag="tb", bufs=2, name=f"tb{i}")
                    nc.vector.tensor_mul(tb[:], qsw[:], sn_t[:, t0:t0 + TCH])
                    rs = pool.tile([128, TCH], bf16, tag="rs", bufs=2, name=f"rs{i}")
                    nc.vector.tensor_add(rs[:], ta[:], tb[:])
                    nc.vector.tensor_mul(dsts[i], rs[:], bc[:])

            # ---- phase 1: q/k/v projection, norm+rope, all SBUF-resident ----
            with tc.tile_pool(name="pj", bufs=1) as pj:
                wq_t = pj.tile([128, NH * NDK * 128], bf16)
                for j in range(4):
                    s = NH * NDK * 128 // 4
                    nc.sync.dma_start(wq_t[:, j * s:(j + 1) * s], wq_ap[:, j * s:(j + 1) * s])
                wk_t = pj.tile([128, NKV * NDK * 128], bf16)
                for j in range(2):
                    s = NKV * NDK * 128 // 2
                    nc.sync.dma_start(wk_t[:, j * s:(j + 1) * s], wk_ap[:, j * s:(j + 1) * s])
                wv_t = pj.tile([128, NKV * NDK * 128], bf16)
                for j in range(2):
                    s = NKV * NDK * 128 // 2
                    nc.sync.dma_start(wv_t[:, j * s:(j + 1) * s], wv_ap[:, j * s:(j + 1) * s])

                acc_roll = [0]

                def chain(w_t, head, xh0, xh1):
                    """Sequential 16-dk matmul chain into one rolling psum bank."""
                    acc = pp.tile([128, 512], f32, tag=f"a{acc_roll[0] % 3}", name="acc")
                    acc_roll[0] += 1
                    for dk in range(NDK):
                        xh = xh0 if dk < 8 else xh1
                        nc.tensor.matmul(
                            acc[:],
                            w_t[:, (head * NDK + dk) * 128:(head * NDK + dk + 1) * 128],
                            xh[:, (dk % 8) * TCH:(dk % 8 + 1) * TCH],
                            start=(dk == 0), stop=(dk == NDK - 1))
                    return acc

                for ch in range(NCH):
                    t0 = ch * TCH
                    # x chunk, in two dk-halves for SBUF thrift
                    xh0 = pj.tile([128, 8 * TCH], bf16, tag="xh0", bufs=2, name="xh0")
                    for dk in range(8):
                        nc.sync.dma_start(xh0[:, dk * TCH:(dk + 1) * TCH],
                                          xt_ap[dk * 128:(dk + 1) * 128, t0:t0 + TCH])
                    xh1 = pj.tile([128, 8 * TCH], bf16, tag="xh1", bufs=2, name="xh1")
                    for dk in range(8):
                        nc.sync.dma_start(xh1[:, dk * TCH:(dk + 1) * TCH],
                                          xt_ap[(8 + dk) * 128:(9 + dk) * 128, t0:t0 + TCH])
                    # q heads, two groups of 4
                    for g in range(2):
                        accs = [chain(wq_t, g * 4 + i, xh0, xh1) for i in range(4)]
                        drain_group(pj, [a[:] for a in accs], qsc_t, t0,
                                    [qT[:, (g * 4 + i) * T + t0:(g * 4 + i) * T + t0 + TCH]
                                     for i in range(4)])
                    # k heads
                    accs = [chain(wk_t, i, xh0, xh1) for i in range(NKV)]
                    drain_group(pj, [a[:] for a in accs], ksc_t, t0,
                                [kT[:, kv * T + t0:kv * T + t0 + TCH] for kv in range(NKV)])
                    # v heads: no norm/rope; transpose into [t,H] blocks via DMA xbar
                    for kv in range(NKV):
                        acc = chain(wv_t, kv, xh0, xh1)
                        vtmp = pj.tile([128, TCH], bf16, tag="vtmp", bufs=2, name="vtmp")
                        nc.scalar.activation(vtmp[:], acc[:], AF.Copy)
                        for j in range(TCH // 128):
                            nc.sync.dma_start(
                                vT[:, kv * T + t0 + j * 128:kv * T + t0 + (j + 1) * 128],
                                vtmp[:, j * 128:(j + 1) * 128], transpose=True)

            # ---- phase 2: attention + fused o_proj ----
            with tc.tile_pool(name="op", bufs=1) as op:
                wo_t = op.tile([128, NH * D], bf16)
                for j in range(4):
                    s = NH * D // 4
                    nc.sync.dma_start(wo_t[:, j * s:(j + 1) * s], wo_ap[:, j * s:(j + 1) * s])

                for qi in range(NQC):
                    q0 = qi * TCH
                    attn_sb = op.tile([128, NH * TCH], bf16, tag="attn", bufs=2, name="attn")
                    nkj = 4 * qi + 4
                    for h in range(NH):
                        kv = h // 2
                        o_ps = pp.tile([128, 512], f32, tag=f"a{5 + h % 2}", name="ops")
                        acc_sb = op.tile([128, TCH], bf16, tag="accsb", bufs=2, name="accsb")

                        def emit_s(kj):
                            m = kj - 4 * qi
                            lo = 128 * m if m > 0 else 0
                            s_ps = pp.tile([128, 512], f32, tag=f"a{kj % 5}", name="sps")
                            nc.tensor.matmul(
                                s_ps[:, lo:512],
                                kT[:, kv * T + kj * 128:kv * T + (kj + 1) * 128],
                                qT[:, h * T + q0 + lo:h * T + q0 + TCH],
                                start=True, stop=True)
                            pt = op.tile([128, TCH], bf16, tag="pt", bufs=3, name="pt")
                            return s_ps, pt, lo, m, kj

                        def emit_drain(s_ps, pt, lo, m, kj):
                            nc.scalar.activation(pt[:, lo:512], s_ps[:, lo:512], AF.Exp)
                            if m >= 0:
                                nc.vector.tensor_mul(pt[:, lo:lo + 128],
                                                     pt[:, lo:lo + 128], cm_t[:])
                            if kj == 0:
                                nc.vector.tensor_copy(acc_sb[:], pt[:])
                            else:
                                nc.vector.tensor_add(acc_sb[:, lo:512],
                                                     acc_sb[:, lo:512], pt[:, lo:512])
                            nc.tensor.matmul(
                                o_ps[:, lo:512],
                                vT[:, kv * T + kj * 128:kv * T + (kj + 1) * 128],
                                pt[:, lo:512],
                                start=(kj == 0), stop=(kj == nkj - 1),
                                skip_group_check=True)

                        prev = None
                        for kj in range(nkj):
                            cur = emit_s(kj)
                            if prev is not None:
                                emit_drain(*prev)
                            prev = cur
                        emit_drain(*prev)

                        # softmax denominator for this (h, qi)
                        rp = 32 * (h % 3)
                        nc.tensor.matmul(rows[rp:rp + 1, :], ones_col_b[:],
                                         acc_sb[:], start=True, stop=True)
                        rden = op.tile([1, TCH], f32, tag="rden", bufs=2, name="rden")
                        nc.vector.reciprocal(rden[:], rows[rp:rp + 1, :])
                        rbc = op.tile([128, TCH], f32, tag="rbc", bufs=2, name="rbc")
                        nc.gpsimd.partition_broadcast(rbc[:], rden[:])
                        nc.vector.tensor_mul(
                            attn_sb[:, h * TCH:(h + 1) * TCH], o_ps[:], rbc[:])

                    # fused o_proj for this 512-row query chunk
                    for dc in range(4):
                        for ti in range(4):
                            ops2 = pp.tile([128, 512], f32,
                                           tag=f"a{5 + (dc * 4 + ti) % 2}", name="ops2")
                            for h in range(NH):
                                nc.tensor.matmul(
                                    ops2[:],
                                    attn_sb[:, h * TCH + ti * 128:h * TCH + (ti + 1) * 128],
                                    wo_t[:, h * D + dc * 512:h * D + (dc + 1) * 512],
                                    start=(h == 0), stop=(h == NH - 1))
                            stg = op.tile([128, 512], bf16, tag="ostg", bufs=3, name="ostg")
                            if (dc * 4 + ti) % 2 == 0:
                                nc.vector.tensor_copy(stg[:], ops2[:])
                            else:
                                nc.scalar.activation(stg[:], ops2[:], AF.Copy)
                            nc.sync.dma_start(
                                out_ap[q0 + ti * 128:q0 + (ti + 1) * 128,
                                       dc * 512:(dc + 1) * 512], stg[:])

    nc.compile()
    return nc


def _pack(w):
    """(nh, D, H) -> (128, nh*NDK*128): col block (h*NDK+dk)*128 = w[h, dk*128:+128, :]."""
    nh = w.shape[0]
    a = w.reshape(nh, NDK, 128, H).transpose(2, 0, 1, 3)
    return np.ascontiguousarray(a.reshape(128, nh * NDK * H)).astype(npbf16)


def _numpy_ref(x, mask, position, qp, kvp, op, qns, kns):
    def rms(v, s):
        var = (v * v).mean(-1, keepdims=True)
        return v / np.sqrt(var + EPS) * (1.0 + s)

    def rope(v, pos):
        ts = THETA ** (np.arange(64, dtype=np.float32) * 2.0 / H)
        ang = pos.astype(np.float32)[:, :, None, None] / ts
        sn, cs = np.sin(ang), np.cos(ang)
        x1, x2 = v[..., :64], v[..., 64:]
        return np.concatenate([x1 * cs - x2 * sn, x2 * cs + x1 * sn], -1)

    q = np.einsum('BTD,NDH->BTNH', x, qp)
    k = np.einsum('BTD,KDH->BTKH', x, kvp[0])
    v = np.einsum('BTD,KDH->BTKH', x, kvp[1])
    q = rope(rms(q, qns), position) * (H ** -0.5)
    k = rope(rms(k, kns), position)
    q = q.transpose(0, 2, 1, 3)
    k = np.repeat(k.transpose(0, 2, 1, 3), NQ // NK, 1)
    v = np.repeat(v.transpose(0, 2, 1, 3), NQ // NK, 1)
    s = np.einsum('BHtD,BHTD->BHtT', q, k) / np.sqrt(np.float32(H))
    s = np.where(mask[:, None], s, np.float32(-2.3819763e+38))
    s = s - s.max(-1, keepdims=True)
    w = np.exp(s)
    w /= w.sum(-1, keepdims=True)
    o = np.einsum('BHtT,BHTD->BHtD', w, v)
    return np.einsum('BNTH,NHD->BTD', o, op).astype(np.float32)


def kernel(**inputs):
    global LAST_EXEC_NS
    x = np.asarray(inputs["x"], np.float32)
    mask = np.asarray(inputs["mask"])
    position = np.asarray(inputs["position"])
    qp = np.asarray(inputs["q_proj"], np.float32)
    kvp = np.asarray(inputs["kv_proj"], np.float32)
    op = np.asarray(inputs["o_proj"], np.float32)
    qns = np.asarray(inputs["q_norm_scale"], np.float32)
    kns = np.asarray(inputs["k_norm_scale"], np.float32)

    tril = np.tril(np.ones((T, T), bool))
    if mask.shape != (B, T, T) or not all(np.array_equal(mask[b], tril) for b in range(B)):
        return _numpy_ref(x, mask, position, qp, kvp, op, qns, kns)

    if "nc" not in _CACHE:
        _CACHE["nc"] = _build()
    nc = _CACHE["nc"]

    halves = []
    for half in range(2):
        halves.append((
            _pack(qp[half * NH:(half + 1) * NH]),
            _pack(kvp[0, half * NKV:(half + 1) * NKV]),
            _pack(kvp[1, half * NKV:(half + 1) * NKV]),
            np.ascontiguousarray(
                op[half * NH:(half + 1) * NH].transpose(1, 0, 2).reshape(128, NH * D)
            ).astype(npbf16),
        ))
    qsc = ((1.0 + qns) / np.sqrt(H)).reshape(128, 1).astype(np.float32)
    ksc = ((1.0 + kns) * np.sqrt(H)).reshape(128, 1).astype(np.float32)
    ts = THETA ** (np.arange(64, dtype=np.float64) * 2.0 / H)
    pidx = np.arange(128)[:, None]
    fidx = np.arange(128)[None, :]
    cm = (fidx >= pidx).astype(npbf16)

    in_maps = []
    for c in range(8):
        b, half = c // 2, c % 2
        wq, wk, wv, wo = halves[half]
        ang = position[b].astype(np.float64)[None, :] / ts[:, None]
        sn = np.sin(ang).astype(np.float32)
        cs = np.cos(ang).astype(np.float32)
        in_maps.append({
            "xt": np.ascontiguousarray(x[b].T).astype(npbf16),
            "wq": wq, "wk": wk, "wv": wv, "wo": wo,
            "cs": np.ascontiguousarray(np.concatenate([cs, cs], 0)).astype(npbf16),
            "sn": np.ascontiguousarray(np.concatenate([-sn, sn], 0)).astype(npbf16),
            "qsc": qsc, "ksc": ksc, "cm": cm,
        })

    if TRACE:
        _install_hook()
    last_err = None
    for _ in range(3):
        try:
            res = bass_utils.run_bass_kernel_spmd(nc, in_maps, list(range(8)), trace=TRACE)
            break
        except Exception as e:  # transient NRT device wedge
            last_err = e
    else:
        raise last_err
    LAST_EXEC_NS = getattr(res, "exec_time_ns", None)

    out = np.empty((B, T, D), np.float32)
    for b in range(B):
        out[b] = (res.results[2 * b]["out"].astype(np.float32)
                  + res.results[2 * b + 1]["out"].astype(np.float32))
    return out


# revision 21
# speedup vs baseline: 1.3213x; 1.3213x over previous
"""MultiHeadAttention (B=4,T=2048,D=2048,NQ=16,NK=8,H=128) on 8 trn2 cores.

Sharding: core c -> batch b=c//2, half=c%2. Each core computes the partial
output for batch b restricted to q-heads [half*8, half*8+8) (kv-heads
[half*4, half*4+4)); host sums the two partials per batch (o_proj
contraction over heads is split across the core pair).

v2: bf16 matmul/vector datapath (PSUM accumulation stays f32), fused
q/k/v projection pass with SBUF-resident q (no DRAM spill), DMA-engine
transposes for V, batched RMSNorm row statistics, f32r broadcast
matmuls, causal-restricted score/exp/PV tiles, software-pipelined
exp/PV attention loop with o_proj fused per 512-row query chunk.
"""
import numpy as np
import concourse.bass as bass
import concourse.tile as tile
from concourse import bacc, mybir
from concourse import bass_utils

B, T, D = 4, 2048, 2048
NQ, NK, H = 16, 8, 128
NH, NKV = 8, 4          # per-core q heads / kv heads
THETA = 10000.0
EPS = 1e-6
TCH = 512               # chunk of T for projections / attention q blocks
NCH = T // TCH
NDK = D // 128
NQC = T // TCH

f32 = mybir.dt.float32
f32r = mybir.dt.float32r
bf16 = mybir.dt.bfloat16
npbf16 = mybir.dt.np(bf16)
AF = mybir.ActivationFunctionType
MUL = mybir.AluOpType.mult

TRACE = False
LAST_EXEC_NS = None
_CACHE = {}


def _install_hook():
    import contextlib, ctypes, sys, types
    if "antenv.axon_hooks" in sys.modules:
        return
    lib = ctypes.CDLL("/opt/axon/libaxon_pjrt.so")
    lib.axon_start_nrt_profile.argtypes = [ctypes.POINTER(ctypes.c_int64), ctypes.c_size_t]
    lib.axon_start_nrt_profile.restype = ctypes.c_int64
    lib.axon_stop_nrt_profile.argtypes = [ctypes.c_char_p]
    lib.axon_stop_nrt_profile.restype = ctypes.c_int64

    @contextlib.contextmanager
    def _hook(output_dir, device_ids):
        import jax
        jax.devices()
        ids = (ctypes.c_int64 * len(device_ids))(*device_ids) if device_ids else None
        rc = lib.axon_start_nrt_profile(ids, len(device_ids) if device_ids else 0)
        if rc != 0:
            raise RuntimeError(f"axon_start_nrt_profile rc={rc}")
        try:
            yield
        finally:
            n = lib.axon_stop_nrt_profile(str(output_dir).encode())
            if n < 0:
                raise RuntimeError(f"axon_stop_nrt_profile rc={n}")

    mod = types.ModuleType("antenv.axon_hooks")
    mod.get_axon_ntff_profile_hook = lambda: _hook
    mod.set_axon_ntff_profile_hook = lambda h: None
    sys.modules["antenv.axon_hooks"] = mod
    bass_utils.upload_artifacts = lambda tmpdir: "local://" + str(tmpdir)


def _build():
    nc = bacc.Bacc("TRN2", target_bir_lowering=False, debug=False, num_devices=8)
    xt_ap = nc.dram_tensor("xt", [D, T], bf16, kind="ExternalInput").ap()
    wq_ap = nc.dram_tensor("wq", [128, NH * NDK * 128], bf16, kind="ExternalInput").ap()
    wk_ap = nc.dram_tensor("wk", [128, NKV * NDK * 128], bf16, kind="ExternalInput").ap()
    wv_ap = nc.dram_tensor("wv", [128, NKV * NDK * 128], bf16, kind="ExternalInput").ap()
    wo_ap = nc.dram_tensor("wo", [128, NH * D], bf16, kind="ExternalInput").ap()
    cs_ap = nc.dram_tensor("cs", [128, T], bf16, kind="ExternalInput").ap()
    sn_ap = nc.dram_tensor("sn", [128, T], bf16, kind="ExternalInput").ap()
    qsc_ap = nc.dram_tensor("qsc", [128, 1], f32, kind="ExternalInput").ap()
    ksc_ap = nc.dram_tensor("ksc", [128, 1], f32, kind="ExternalInput").ap()
    cm_ap = nc.dram_tensor("cm", [128, 128], bf16, kind="ExternalInput").ap()
    out_ap = nc.dram_tensor("out", [T, D], bf16, kind="ExternalOutput").ap()

    with tile.TileContext(nc) as tc:
        with tc.tile_pool(name="mp", bufs=1) as mp, \
             tc.tile_pool(name="pp", bufs=1, space="PSUM") as pp:
            # ---- persistent tiles ----
            cs_t = mp.tile([128, T], bf16)
            nc.sync.dma_start(cs_t[:], cs_ap[:])
            sn_t = mp.tile([128, T], bf16)
            nc.sync.dma_start(sn_t[:], sn_ap[:])
            qsc_t = mp.tile([128, 1], f32)
            nc.sync.dma_start(qsc_t[:], qsc_ap[:])
            ksc_t = mp.tile([128, 1], f32)
            nc.sync.dma_start(ksc_t[:], ksc_ap[:])
            cm_t = mp.tile([128, 128], bf16)
            nc.sync.dma_start(cm_t[:], cm_ap[:])
            eps_t = mp.tile([1, 1], f32)
            nc.vector.memset(eps_t[:], EPS)
            ones_col_b = mp.tile([128, 1], bf16)
            nc.vector.memset(ones_col_b[:], 1.0)
            ones_row_b = mp.tile([1, 128], bf16)
            nc.vector.memset(ones_row_b[:], 1.0)

            qT = mp.tile([128, NH * T], bf16)     # 32KB/part
            kT = mp.tile([128, NKV * T], bf16)    # 16KB
            vT = mp.tile([128, NKV * T], bf16)    # 16KB

            # PSUM bank for softmax denominator rows (attention phase).
            rows = pp.tile([128, 512], f32, tag="rows")

            def drain_group(pool, accs, sc_t, t0, dsts):
                """accs: psum [128,512] f32 -> RMSNorm*(gain) + RoPE -> dsts bf16."""
                n = len(accs)
                for i in range(n):
                    sq = pool.tile([128, TCH], bf16, tag="sq", bufs=2, name=f"sq{i}")
                    nc.scalar.activation(sq[:], accs[i], AF.Square)
                    row = pp.tile([1, 512], f32, tag=f"a{3 + i % 2}", name=f"row{i}")
                    nc.tensor.matmul(row[:], ones_col_b[:], sq[:], start=True, stop=True)
                    rinv = pool.tile([1, TCH], f32, tag="rinv", bufs=2, name=f"rinv{i}")
                    nc.vector.reciprocal_approx_fast(rinv[:], row[:])
                    rstd = pool.tile([1, TCH], bf16, tag="rstd", bufs=2, name=f"rstd{i}")
                    nc.scalar.activation(rstd[:], rinv[:], AF.Sqrt)
                    bc = pp.tile([128, 512], f32, tag=f"a{3 + (i + 1) % 2}", name=f"bc{i}")
                    nc.tensor.matmul(bc[:], ones_row_b[:], rstd[:],
                                     start=True, stop=True)
                    # qn0 = acc * gain (per-partition); rstd applied after rope
                    # (valid: rstd is per-column, invariant under the half-swap)
                    qn = pool.tile([128, TCH], bf16, tag="qn", bufs=2, name=f"qn{i}")
                    nc.scalar.activation(qn[:], accs[i], AF.Copy, scale=sc_t[:])
                    qsw = pool.tile([128, TCH], bf16, tag="qsw", bufs=2, name=f"qsw{i}")
                    nc.sync.dma_start(qsw[0:64, :], qn[64:128, :])
                    nc.sync.dma_start(qsw[64:128, :], qn[0:64, :])
                    ta = pool.tile([128, TCH], bf16, tag="ta", bufs=2, name=f"ta{i}")
                    nc.vector.tensor_mul(ta[:], qn[:], cs_t[:, t0:t0 + TCH])
                    tb = pool.tile([128, TCH], bf16, tag="tb", bufs=2, name=f"tb{i}")
                    nc.vector.tensor_mul(tb[:], qsw[:], sn_t[:, t0:t0 + TCH])
                    rs = pool.tile([128, TCH], bf16, tag="rs", bufs=2, name=f"rs{i}")
                    nc.vector.tensor_add(rs[:], ta[:], tb[:])
                    nc.vector.tensor_mul(dsts[i], rs[:], bc[:])

            # ---- phase 1: q/k/v projection, norm+rope, all SBUF-resident ----
            with tc.tile_pool(name="pj", bufs=1) as pj:
                wq_t = pj.tile([128, NH * NDK * 128], bf16)
                for j in range(4):
                    s = NH * NDK * 128 // 4
                    nc.sync.dma_start(wq_t[:, j * s:(j + 1) * s], wq_ap[:, j * s:(j + 1) * s])
                wk_t = pj.tile([128, NKV * NDK * 128], bf16)
                for j in range(2):
                    s = NKV * NDK * 128 // 2
                    nc.sync.dma_start(wk_t[:, j * s:(j + 1) * s], wk_ap[:, j * s:(j + 1) * s])
                wv_t = pj.tile([128, NKV * NDK * 128], bf16)
                for j in range(2):
                    s = NKV * NDK * 128 // 2
                    nc.sync.dma_start(wv_t[:, j * s:(j + 1) * s], wv_ap[:, j * s:(j + 1) * s])

                acc_roll = [0]

                def chain(w_t, head, xh0, xh1):
                    """Sequential 16-dk matmul chain into one rolling psum bank."""
                    acc = pp.tile([128, 512], f32, tag=f"a{acc_roll[0] % 3}", name="acc")
                    acc_roll[0] += 1
                    for dk in range(NDK):
                        xh = xh0 if dk < 8 else xh1
                        nc.tensor.matmul(
                            acc[:],
                            w_t[:, (head * NDK + dk) * 128:(head * NDK + dk + 1) * 128],
                            xh[:, (dk % 8) * TCH:(dk % 8 + 1) * TCH],
                            start=(dk == 0), stop=(dk == NDK - 1))
                    return acc

                for ch in range(NCH):
                    t0 = ch * TCH
                    # x chunk, in two dk-halves for SBUF thrift
                    xh0 = pj.tile([128, 8 * TCH], bf16, tag="xh0", bufs=2, name="xh0")
                    for dk in range(8):
                        nc.sync.dma_start(xh0[:, dk * TCH:(dk + 1) * TCH],
                                          xt_ap[dk * 128:(dk + 1) * 128, t0:t0 + TCH])
                    xh1 = pj.tile([128, 8 * TCH], bf16, tag="xh1", bufs=2, name="xh1")
                    for dk in range(8):
                        nc.sync.dma_start(xh1[:, dk * TCH:(dk + 1) * TCH],
                                          xt_ap[(8 + dk) * 128:(9 + dk) * 128, t0:t0 + TCH])
                    # q heads, two groups of 4
                    for g in range(2):
                        accs = [chain(wq_t, g * 4 + i, xh0, xh1) for i in range(4)]
                        drain_group(pj, [a[:] for a in accs], qsc_t, t0,
                                    [qT[:, (g * 4 + i) * T + t0:(g * 4 + i) * T + t0 + TCH]
                                     for i in range(4)])
                    # k heads
                    accs = [chain(wk_t, i, xh0, xh1) for i in range(NKV)]
                    drain_group(pj, [a[:] for a in accs], ksc_t, t0,
                                [kT[:, kv * T + t0:kv * T + t0 + TCH] for kv in range(NKV)])
                    # v heads: no norm/rope; transpose into [t,H] blocks via DMA xbar
                    for kv in range(NKV):
                        acc = chain(wv_t, kv, xh0, xh1)
                        vtmp = pj.tile([128, TCH], bf16, tag="vtmp", bufs=2, name="vtmp")
                        nc.scalar.activation(vtmp[:], acc[:], AF.Copy)
                        for j in range(TCH // 128):
                            nc.sync.dma_start(
                                vT[:, kv * T + t0 + j * 128:kv * T + t0 + (j + 1) * 128],
                                vtmp[:, j * 128:(j + 1) * 128], transpose=True)

            # ---- phase 2: attention + fused o_proj ----
            with tc.tile_pool(name="op", bufs=1) as op:
                wo_t = op.tile([128, NH * D], bf16)
                for j in range(4):
                    s = NH * D // 4
                    nc.sync.dma_start(wo_t[:, j * s:(j + 1) * s], wo_ap[:, j * s:(j + 1) * s])

                for qi in range(NQC):
                    q0 = qi * TCH
                    attn_sb = op.tile([128, NH * TCH], bf16, tag="attn", bufs=2, name="attn")
                    nkj = 4 * qi + 4
                    for h in range(NH):
                        kv = h // 2
                        o_ps = pp.tile([128, 512], f32, tag=f"a{5 + h % 2}", name="ops")
                        acc_sb = op.tile([128, TCH], bf16, tag="accsb", bufs=2, name="accsb")

                        def emit_s(kj):
                            m = kj - 4 * qi
                            lo = 128 * m if m > 0 else 0
                            s_ps = pp.tile([128, 512], f32, tag=f"a{kj % 5}", name="sps")
                            nc.tensor.matmul(
                                s_ps[:, lo:512],
                                kT[:, kv * T + kj * 128:kv * T + (kj + 1) * 128],
                                qT[:, h * T + q0 + lo:h * T + q0 + TCH],
                                start=True, stop=True)
                            pt = op.tile([128, TCH], bf16, tag="pt", bufs=3, name="pt")
                            return s_ps, pt, lo, m, kj

                        def emit_drain(s_ps, pt, lo, m, kj):
                            nc.scalar.activation(pt[:, lo:512], s_ps[:, lo:512], AF.Exp)
                            if m >= 0:
                                nc.vector.tensor_mul(pt[:, lo:lo + 128],
                                                     pt[:, lo:lo + 128], cm_t[:])
                            if kj == 0:
                                nc.vector.tensor_copy(acc_sb[:], pt[:])
                            else:
                                nc.vector.tensor_add(acc_sb[:, lo:512],
                                                     acc_sb[:, lo:512], pt[:, lo:512])
                            nc.tensor.matmul(
                                o_ps[:, lo:512],
                                vT[:, kv * T + kj * 128:kv * T + (kj + 1) * 128],
                                pt[:, lo:512],
                                start=(kj == 0), stop=(kj == nkj - 1),
                                skip_group_check=True)

                        prev = None
                        for kj in range(nkj):
                            cur = emit_s(kj)
                            if prev is not None:
                                emit_drain(*prev)
                            prev = cur
                        emit_drain(*prev)

                        # softmax denominator for this (h, qi).
                        # NB: reciprocal_approx_fast corrupts results when its
                        # input sits at a nonzero base partition -> keep row 0.
                        nc.tensor.matmul(rows[0:1, :], ones_col_b[:],
                                         acc_sb[:], start=True, stop=True)
                        rden = op.tile([1, TCH], f32, tag="rden", bufs=2, name="rden")
                        nc.vector.reciprocal_approx_fast(rden[:], rows[0:1, :])
                        rbc = op.tile([128, TCH], f32, tag="rbc", bufs=2, name="rbc")
                        nc.gpsimd.partition_broadcast(rbc[:], rden[:])
                        nc.vector.tensor_mul(
                            attn_sb[:, h * TCH:(h + 1) * TCH], o_ps[:], rbc[:])

                    # fused o_proj for this 512-row query chunk
                    for dc in range(4):
                        for ti in range(4):
                            ops2 = pp.tile([128, 512], f32,
                                           tag=f"a{5 + (dc * 4 + ti) % 2}", name="ops2")
                            for h in range(NH):
                                nc.tensor.matmul(
                                    ops2[:],
                                    attn_sb[:, h * TCH + ti * 128:h * TCH + (ti + 1) * 128],
                                    wo_t[:, h * D + dc * 512:h * D + (dc + 1) * 512],
                                    start=(h == 0), stop=(h == NH - 1))
                            stg = op.tile([128, 512], bf16, tag="ostg", bufs=3, name="ostg")
                            if (dc * 4 + ti) % 2 == 0:
                                nc.vector.tensor_copy(stg[:], ops2[:])
                            else:
                                nc.scalar.activation(stg[:], ops2[:], AF.Copy)
                            nc.sync.dma_start(
                                out_ap[q0 + ti * 128:q0 + (ti + 1) * 128,
                                       dc * 512:(dc + 1) * 512], stg[:])

    nc.compile()
    return nc


def _pack(w):
    """(nh, D, H) -> (128, nh*NDK*128): col block (h*NDK+dk)*128 = w[h, dk*128:+128, :]."""
    nh = w.shape[0]
    a = w.reshape(nh, NDK, 128, H).transpose(2, 0, 1, 3)
    return np.ascontiguousarray(a.reshape(128, nh * NDK * H)).astype(npbf16)


def _numpy_ref(x, mask, position, qp, kvp, op, qns, kns):
    def rms(v, s):
        var = (v * v).mean(-1, keepdims=True)
        return v / np.sqrt(var + EPS) * (1.0 + s)

    def rope(v, pos):
        ts = THETA ** (np.arange(64, dtype=np.float32) * 2.0 / H)
        ang = pos.astype(np.float32)[:, :, None, None] / ts
        sn, cs = np.sin(ang), np.cos(ang)
        x1, x2 = v[..., :64], v[..., 64:]
        return np.concatenate([x1 * cs - x2 * sn, x2 * cs + x1 * sn], -1)

    q = np.einsum('BTD,NDH->BTNH', x, qp)
    k = np.einsum('BTD,KDH->BTKH', x, kvp[0])
    v = np.einsum('BTD,KDH->BTKH', x, kvp[1])
    q = rope(rms(q, qns), position) * (H ** -0.5)
    k = rope(rms(k, kns), position)
    q = q.transpose(0, 2, 1, 3)
    k = np.repeat(k.transpose(0, 2, 1, 3), NQ // NK, 1)
    v = np.repeat(v.transpose(0, 2, 1, 3), NQ // NK, 1)
    s = np.einsum('BHtD,BHTD->BHtT', q, k) / np.sqrt(np.float32(H))
    s = np.where(mask[:, None], s, np.float32(-2.3819763e+38))
    s = s - s.max(-1, keepdims=True)
    w = np.exp(s)
    w /= w.sum(-1, keepdims=True)
    o = np.einsum('BHtT,BHTD->BHtD', w, v)
    return np.einsum('BNTH,NHD->BTD', o, op).astype(np.float32)


def kernel(**inputs):
    global LAST_EXEC_NS
    x = np.asarray(inputs["x"], np.float32)
    mask = np.asarray(inputs["mask"])
    position = np.asarray(inputs["position"])
    qp = np.asarray(inputs["q_proj"], np.float32)
    kvp = np.asarray(inputs["kv_proj"], np.float32)
    op = np.asarray(inputs["o_proj"], np.float32)
    qns = np.asarray(inputs["q_norm_scale"], np.float32)
    kns = np.asarray(inputs["k_norm_scale"], np.float32)

    tril = np.tril(np.ones((T, T), bool))
    if mask.shape != (B, T, T) or not all(np.array_equal(mask[b], tril) for b in range(B)):
        return _numpy_ref(x, mask, position, qp, kvp, op, qns, kns)

    if "nc" not in _CACHE:
        _CACHE["nc"] = _build()
    nc = _CACHE["nc"]

    halves = []
    for half in range(2):
        halves.append((
            _pack(qp[half * NH:(half + 1) * NH]),
            _pack(kvp[0, half * NKV:(half + 1) * NKV]),
            _pack(kvp[1, half * NKV:(half + 1) * NKV]),
            np.ascontiguousarray(
                op[half * NH:(half + 1) * NH].transpose(1, 0, 2).reshape(128, NH * D)
            ).astype(npbf16),
        ))
    qsc = ((1.0 + qns) / np.sqrt(H)).reshape(128, 1).astype(np.float32)
    ksc = ((1.0 + kns) * np.sqrt(H)).reshape(128, 1).astype(np.float32)
    ts = THETA ** (np.arange(64, dtype=np.float64) * 2.0 / H)
    pidx = np.arange(128)[:, None]
    fidx = np.arange(128)[None, :]
    cm = (fidx >= pidx).astype(npbf16)

    in_maps = []
    for c in range(8):
        b, half = c // 2, c % 2
        wq, wk, wv, wo = halves[half]
        ang = position[b].astype(np.float64)[None, :] / ts[:, None]
        sn = np.sin(ang).astype(np.float32)
        cs = np.cos(ang).astype(np.float32)
        in_maps.append({
            "xt": np.ascontiguousarray(x[b].T).astype(npbf16),
            "wq": wq, "wk": wk, "wv": wv, "wo": wo,
            "cs": np.ascontiguousarray(np.concatenate([cs, cs], 0)).astype(npbf16),
            "sn": np.ascontiguousarray(np.concatenate([-sn, sn], 0)).astype(npbf16),
            "qsc": qsc, "ksc": ksc, "cm": cm,
        })

    if TRACE:
        _install_hook()
    last_err = None
    for _ in range(3):
        try:
            res = bass_utils.run_bass_kernel_spmd(nc, in_maps, list(range(8)), trace=TRACE)
            break
        except Exception as e:  # transient NRT device wedge
            last_err = e
    else:
        raise last_err
    LAST_EXEC_NS = getattr(res, "exec_time_ns", None)

    out = np.empty((B, T, D), np.float32)
    for b in range(B):
        out[b] = (res.results[2 * b]["out"].astype(np.float32)
                  + res.results[2 * b + 1]["out"].astype(np.float32))
    return out


# revision 22
# speedup vs baseline: 1.3354x; 1.0107x over previous
"""MultiHeadAttention (B=4,T=2048,D=2048,NQ=16,NK=8,H=128) on 8 trn2 cores.

Sharding: core c -> batch b=c//2, half=c%2. Each core computes the partial
output for batch b restricted to q-heads [half*8, half*8+8) (kv-heads
[half*4, half*4+4)); host sums the two partials per batch (o_proj
contraction over heads is split across the core pair).

v2: bf16 matmul/vector datapath (PSUM accumulation stays f32), fused
q/k/v projection pass with SBUF-resident q (no DRAM spill), DMA-engine
transposes for V, batched RMSNorm row statistics, f32r broadcast
matmuls, causal-restricted score/exp/PV tiles, software-pipelined
exp/PV attention loop with o_proj fused per 512-row query chunk.
"""
import numpy as np
import concourse.bass as bass
import concourse.tile as tile
from concourse import bacc, mybir
from concourse import bass_utils

B, T, D = 4, 2048, 2048
NQ, NK, H = 16, 8, 128
NH, NKV = 8, 4          # per-core q heads / kv heads
THETA = 10000.0
EPS = 1e-6
TCH = 512               # chunk of T for projections / attention q blocks
NCH = T // TCH
NDK = D // 128
NQC = T // TCH

f32 = mybir.dt.float32
f32r = mybir.dt.float32r
bf16 = mybir.dt.bfloat16
npbf16 = mybir.dt.np(bf16)
AF = mybir.ActivationFunctionType
MUL = mybir.AluOpType.mult

TRACE = False
LAST_EXEC_NS = None
_CACHE = {}


def _install_hook():
    import contextlib, ctypes, sys, types
    if "antenv.axon_hooks" in sys.modules:
        return
    lib = ctypes.CDLL("/opt/axon/libaxon_pjrt.so")
    lib.axon_start_nrt_profile.argtypes = [ctypes.POINTER(ctypes.c_int64), ctypes.c_size_t]
    lib.axon_start_nrt_profile.restype = ctypes.c_int64
    lib.axon_stop_nrt_profile.argtypes = [ctypes.c_char_p]
    lib.axon_stop_nrt_profile.restype = ctypes.c_int64

    @contextlib.contextmanager
    def _hook(output_dir, device_ids):
        import jax
        jax.devices()
        ids = (ctypes.c_int64 * len(device_ids))(*device_ids) if device_ids else None
        rc = lib.axon_start_nrt_profile(ids, len(device_ids) if device_ids else 0)
        if rc != 0:
            raise RuntimeError(f"axon_start_nrt_profile rc={rc}")
        try:
            yield
        finally:
            n = lib.axon_stop_nrt_profile(str(output_dir).encode())
            if n < 0:
                raise RuntimeError(f"axon_stop_nrt_profile rc={n}")

    mod = types.ModuleType("antenv.axon_hooks")
    mod.get_axon_ntff_profile_hook = lambda: _hook
    mod.set_axon_ntff_profile_hook = lambda h: None
    sys.modules["antenv.axon_hooks"] = mod
    bass_utils.upload_artifacts = lambda tmpdir: "local://" + str(tmpdir)


def _build():
    nc = bacc.Bacc("TRN2", target_bir_lowering=False, debug=False, num_devices=8)
    xt_ap = nc.dram_tensor("xt", [D, T], bf16, kind="ExternalInput").ap()
    wq_ap = nc.dram_tensor("wq", [128, NH * NDK * 128], bf16, kind="ExternalInput").ap()
    wk_ap = nc.dram_tensor("wk", [128, NKV * NDK * 128], bf16, kind="ExternalInput").ap()
    wv_ap = nc.dram_tensor("wv", [128, NKV * NDK * 128], bf16, kind="ExternalInput").ap()
    wo_ap = nc.dram_tensor("wo", [128, NH * D], bf16, kind="ExternalInput").ap()
    cs_ap = nc.dram_tensor("cs", [128, T], bf16, kind="ExternalInput").ap()
    sn_ap = nc.dram_tensor("sn", [128, T], bf16, kind="ExternalInput").ap()
    qsc_ap = nc.dram_tensor("qsc", [128, 1], f32, kind="ExternalInput").ap()
    ksc_ap = nc.dram_tensor("ksc", [128, 1], f32, kind="ExternalInput").ap()
    cm_ap = nc.dram_tensor("cm", [128, 128], bf16, kind="ExternalInput").ap()
    out_ap = nc.dram_tensor("out", [T, D], bf16, kind="ExternalOutput").ap()

    with tile.TileContext(nc) as tc:
        with tc.tile_pool(name="mp", bufs=1) as mp, \
             tc.tile_pool(name="pp", bufs=1, space="PSUM") as pp:
            # ---- persistent tiles ----
            cs_t = mp.tile([128, T], bf16)
            nc.sync.dma_start(cs_t[:], cs_ap[:])
            sn_t = mp.tile([128, T], bf16)
            nc.sync.dma_start(sn_t[:], sn_ap[:])
            qsc_t = mp.tile([128, 1], f32)
            nc.sync.dma_start(qsc_t[:], qsc_ap[:])
            ksc_t = mp.tile([128, 1], f32)
            nc.sync.dma_start(ksc_t[:], ksc_ap[:])
            cm_t = mp.tile([128, 128], bf16)
            nc.sync.dma_start(cm_t[:], cm_ap[:])
            eps_t = mp.tile([1, 1], f32)
            nc.vector.memset(eps_t[:], EPS)
            ones_col_b = mp.tile([128, 1], bf16)
            nc.vector.memset(ones_col_b[:], 1.0)
            ones_row_b = mp.tile([1, 128], bf16)
            nc.vector.memset(ones_row_b[:], 1.0)

            qT = mp.tile([128, NH * T], bf16)     # 32KB/part
            kT = mp.tile([128, NKV * T], bf16)    # 16KB
            vT = mp.tile([128, NKV * T], bf16)    # 16KB

            # PSUM bank for softmax denominator rows (attention phase).
            rows = pp.tile([128, 512], f32, tag="rows")

            def drain_group(pool, accs, sc_t, t0, dsts):
                """accs: psum [128,512] f32 -> RMSNorm*(gain) + RoPE -> dsts bf16."""
                n = len(accs)
                for i in range(n):
                    sq = pool.tile([128, TCH], bf16, tag="sq", bufs=2, name=f"sq{i}")
                    nc.scalar.activation(sq[:], accs[i], AF.Square)
                    row = pp.tile([1, 512], f32, tag=f"a{3 + i % 2}", name=f"row{i}")
                    nc.tensor.matmul(row[:], ones_col_b[:], sq[:], start=True, stop=True)
                    rinv = pool.tile([1, TCH], f32, tag="rinv", bufs=2, name=f"rinv{i}")
                    nc.vector.reciprocal_approx_fast(rinv[:], row[:])
                    rstd = pool.tile([1, TCH], bf16, tag="rstd", bufs=2, name=f"rstd{i}")
                    nc.scalar.activation(rstd[:], rinv[:], AF.Sqrt)
                    bc = pp.tile([128, 512], f32, tag=f"a{3 + (i + 1) % 2}", name=f"bc{i}")
                    nc.tensor.matmul(bc[:], ones_row_b[:], rstd[:],
                                     start=True, stop=True)
                    # qn0 = acc * gain (per-partition); rstd applied after rope
                    # (valid: rstd is per-column, invariant under the half-swap)
                    qn = pool.tile([128, TCH], bf16, tag="qn", bufs=2, name=f"qn{i}")
                    nc.scalar.activation(qn[:], accs[i], AF.Copy, scale=sc_t[:])
                    qsw = pool.tile([128, TCH], bf16, tag="qsw", bufs=2, name=f"qsw{i}")
                    nc.sync.dma_start(qsw[0:64, :], qn[64:128, :])
                    nc.sync.dma_start(qsw[64:128, :], qn[0:64, :])
                    ta = pool.tile([128, TCH], bf16, tag="ta", bufs=2, name=f"ta{i}")
                    nc.vector.tensor_mul(ta[:], qn[:], cs_t[:, t0:t0 + TCH])
                    tb = pool.tile([128, TCH], bf16, tag="tb", bufs=2, name=f"tb{i}")
                    nc.vector.tensor_mul(tb[:], qsw[:], sn_t[:, t0:t0 + TCH])
                    rs = pool.tile([128, TCH], bf16, tag="rs", bufs=2, name=f"rs{i}")
                    nc.vector.tensor_add(rs[:], ta[:], tb[:])
                    nc.vector.tensor_mul(dsts[i], rs[:], bc[:])

            # ---- phase 1: q/k/v projection, norm+rope, all SBUF-resident ----
            with tc.tile_pool(name="pj", bufs=1) as pj:
                wq_t = pj.tile([128, NH * NDK * 128], bf16)
                for j in range(4):
                    s = NH * NDK * 128 // 4
                    nc.sync.dma_start(wq_t[:, j * s:(j + 1) * s], wq_ap[:, j * s:(j + 1) * s])
                wk_t = pj.tile([128, NKV * NDK * 128], bf16)
                for j in range(2):
                    s = NKV * NDK * 128 // 2
                    nc.sync.dma_start(wk_t[:, j * s:(j + 1) * s], wk_ap[:, j * s:(j + 1) * s])
                wv_t = pj.tile([128, NKV * NDK * 128], bf16)
                for j in range(2):
                    s = NKV * NDK * 128 // 2
                    nc.sync.dma_start(wv_t[:, j * s:(j + 1) * s], wv_ap[:, j * s:(j + 1) * s])

                acc_roll = [0]

                def chain(w_t, head, xh0, xh1):
                    """Sequential 16-dk matmul chain into one rolling psum bank."""
                    acc = pp.tile([128, 512], f32, tag=f"a{acc_roll[0] % 3}", name="acc")
                    acc_roll[0] += 1
                    for dk in range(NDK):
                        xh = xh0 if dk < 8 else xh1
                        nc.tensor.matmul(
                            acc[:],
                            w_t[:, (head * NDK + dk) * 128:(head * NDK + dk + 1) * 128],
                            xh[:, (dk % 8) * TCH:(dk % 8 + 1) * TCH],
                            start=(dk == 0), stop=(dk == NDK - 1))
                    return acc

                def load_xh(ch):
                    t0 = ch * TCH
                    xh0 = pj.tile([128, 8 * TCH], bf16, tag="xh0", bufs=2, name="xh0")
                    for dk in range(8):
                        nc.sync.dma_start(xh0[:, dk * TCH:(dk + 1) * TCH],
                                          xt_ap[dk * 128:(dk + 1) * 128, t0:t0 + TCH])
                    xh1 = pj.tile([128, 8 * TCH], bf16, tag="xh1", bufs=2, name="xh1")
                    for dk in range(8):
                        nc.sync.dma_start(xh1[:, dk * TCH:(dk + 1) * TCH],
                                          xt_ap[(8 + dk) * 128:(9 + dk) * 128, t0:t0 + TCH])
                    return xh0, xh1

                xhs = load_xh(0)
                for ch in range(NCH):
                    t0 = ch * TCH
                    xh0, xh1 = xhs
                    # prefetch next chunk's x before this chunk's drains/V
                    # DMAs enter the in-order SP queue
                    if ch + 1 < NCH:
                        xhs = load_xh(ch + 1)
                    # q heads, two groups of 4
                    for g in range(2):
                        accs = [chain(wq_t, g * 4 + i, xh0, xh1) for i in range(4)]
                        drain_group(pj, [a[:] for a in accs], qsc_t, t0,
                                    [qT[:, (g * 4 + i) * T + t0:(g * 4 + i) * T + t0 + TCH]
                                     for i in range(4)])
                    # k heads
                    accs = [chain(wk_t, i, xh0, xh1) for i in range(NKV)]
                    drain_group(pj, [a[:] for a in accs], ksc_t, t0,
                                [kT[:, kv * T + t0:kv * T + t0 + TCH] for kv in range(NKV)])
                    # v heads: no norm/rope; transpose into [t,H] blocks via DMA xbar
                    for kv in range(NKV):
                        acc = chain(wv_t, kv, xh0, xh1)
                        vtmp = pj.tile([128, TCH], bf16, tag="vtmp", bufs=2, name="vtmp")
                        nc.scalar.activation(vtmp[:], acc[:], AF.Copy)
                        for j in range(TCH // 128):
                            nc.sync.dma_start(
                                vT[:, kv * T + t0 + j * 128:kv * T + t0 + (j + 1) * 128],
                                vtmp[:, j * 128:(j + 1) * 128], transpose=True)

            # ---- phase 2: attention + fused o_proj ----
            with tc.tile_pool(name="op", bufs=1) as op:
                wo_t = op.tile([128, NH * D], bf16)
                for j in range(4):
                    s = NH * D // 4
                    nc.sync.dma_start(wo_t[:, j * s:(j + 1) * s], wo_ap[:, j * s:(j + 1) * s])

                for qi in range(NQC):
                    q0 = qi * TCH
                    attn_sb = op.tile([128, NH * TCH], bf16, tag="attn", bufs=2, name="attn")
                    nkj = 4 * qi + 4
                    for h in range(NH):
                        kv = h // 2
                        o_ps = pp.tile([128, 512], f32, tag=f"a{5 + h % 2}", name="ops")
                        acc_sb = op.tile([128, TCH], bf16, tag="accsb", bufs=2, name="accsb")

                        def emit_s(kj):
                            m = kj - 4 * qi
                            lo = 128 * m if m > 0 else 0
                            s_ps = pp.tile([128, 512], f32, tag=f"a{kj % 5}", name="sps")
                            nc.tensor.matmul(
                                s_ps[:, lo:512],
                                kT[:, kv * T + kj * 128:kv * T + (kj + 1) * 128],
                                qT[:, h * T + q0 + lo:h * T + q0 + TCH],
                                start=True, stop=True)
                            pt = op.tile([128, TCH], bf16, tag="pt", bufs=3, name="pt")
                            return s_ps, pt, lo, m, kj

                        def emit_drain(s_ps, pt, lo, m, kj):
                            nc.scalar.activation(pt[:, lo:512], s_ps[:, lo:512], AF.Exp)
                            if m >= 0:
                                nc.vector.tensor_mul(pt[:, lo:lo + 128],
                                                     pt[:, lo:lo + 128], cm_t[:])
                            if kj == 0:
                                nc.vector.tensor_copy(acc_sb[:], pt[:])
                            else:
                                nc.vector.tensor_add(acc_sb[:, lo:512],
                                                     acc_sb[:, lo:512], pt[:, lo:512])
                            nc.tensor.matmul(
                                o_ps[:, lo:512],
                                vT[:, kv * T + kj * 128:kv * T + (kj + 1) * 128],
                                pt[:, lo:512],
                                start=(kj == 0), stop=(kj == nkj - 1),
                                skip_group_check=True)

                        prev = None
                        for kj in range(nkj):
                            cur = emit_s(kj)
                            if prev is not None:
                                emit_drain(*prev)
                            prev = cur
                        emit_drain(*prev)

                        # softmax denominator for this (h, qi).
                        # NB: reciprocal_approx_fast corrupts results when its
                        # input sits at a nonzero base partition -> keep row 0.
                        nc.tensor.matmul(rows[0:1, :], ones_col_b[:],
                                         acc_sb[:], start=True, stop=True)
                        rden = op.tile([1, TCH], f32, tag="rden", bufs=2, name="rden")
                        nc.vector.reciprocal_approx_fast(rden[:], rows[0:1, :])
                        rbc = op.tile([128, TCH], f32, tag="rbc", bufs=2, name="rbc")
                        nc.gpsimd.partition_broadcast(rbc[:], rden[:])
                        nc.vector.tensor_mul(
                            attn_sb[:, h * TCH:(h + 1) * TCH], o_ps[:], rbc[:])

                    # fused o_proj for this 512-row query chunk
                    for dc in range(4):
                        for ti in range(4):
                            ops2 = pp.tile([128, 512], f32,
                                           tag=f"a{5 + (dc * 4 + ti) % 2}", name="ops2")
                            for h in range(NH):
                                nc.tensor.matmul(
                                    ops2[:],
                                    attn_sb[:, h * TCH + ti * 128:h * TCH + (ti + 1) * 128],
                                    wo_t[:, h * D + dc * 512:h * D + (dc + 1) * 512],
                                    start=(h == 0), stop=(h == NH - 1))
                            stg = op.tile([128, 512], bf16, tag="ostg", bufs=3, name="ostg")
                            if (dc * 4 + ti) % 2 == 0:
                                nc.vector.tensor_copy(stg[:], ops2[:])
                            else:
                                nc.scalar.activation(stg[:], ops2[:], AF.Copy)
                            nc.sync.dma_start(
                                out_ap[q0 + ti * 128:q0 + (ti + 1) * 128,
                                       dc * 512:(dc + 1) * 512], stg[:])

    nc.compile()
    return nc


def _pack(w):
    """(nh, D, H) -> (128, nh*NDK*128): col block (h*NDK+dk)*128 = w[h, dk*128:+128, :]."""
    nh = w.shape[0]
    a = w.reshape(nh, NDK, 128, H).transpose(2, 0, 1, 3)
    return np.ascontiguousarray(a.reshape(128, nh * NDK * H)).astype(npbf16)


def _numpy_ref(x, mask, position, qp, kvp, op, qns, kns):
    def rms(v, s):
        var = (v * v).mean(-1, keepdims=True)
        return v / np.sqrt(var + EPS) * (1.0 + s)

    def rope(v, pos):
        ts = THETA ** (np.arange(64, dtype=np.float32) * 2.0 / H)
        ang = pos.astype(np.float32)[:, :, None, None] / ts
        sn, cs = np.sin(ang), np.cos(ang)
        x1, x2 = v[..., :64], v[..., 64:]
        return np.concatenate([x1 * cs - x2 * sn, x2 * cs + x1 * sn], -1)

    q = np.einsum('BTD,NDH->BTNH', x, qp)
    k = np.einsum('BTD,KDH->BTKH', x, kvp[0])
    v = np.einsum('BTD,KDH->BTKH', x, kvp[1])
    q = rope(rms(q, qns), position) * (H ** -0.5)
    k = rope(rms(k, kns), position)
    q = q.transpose(0, 2, 1, 3)
    k = np.repeat(k.transpose(0, 2, 1, 3), NQ // NK, 1)
    v = np.repeat(v.transpose(0, 2, 1, 3), NQ // NK, 1)
    s = np.einsum('BHtD,BHTD->BHtT', q, k) / np.sqrt(np.float32(H))
    s = np.where(mask[:, None], s, np.float32(-2.3819763e+38))
    s = s - s.max(-1, keepdims=True)
    w = np.exp(s)
    w /= w.sum(-1, keepdims=True)
    o = np.einsum('BHtT,BHTD->BHtD', w, v)
    return np.einsum('BNTH,NHD->BTD', o, op).astype(np.float32)


def kernel(**inputs):
    global LAST_EXEC_NS
    x = np.asarray(inputs["x"], np.float32)
    mask = np.asarray(inputs["mask"])
    position = np.asarray(inputs["position"])
    qp = np.asarray(inputs["q_proj"], np.float32)
    kvp = np.asarray(inputs["kv_proj"], np.float32)
    op = np.asarray(inputs["o_proj"], np.float32)
    qns = np.asarray(inputs["q_norm_scale"], np.float32)
    kns = np.asarray(inputs["k_norm_scale"], np.float32)

    tril = np.tril(np.ones((T, T), bool))
    if mask.shape != (B, T, T) or not all(np.array_equal(mask[b], tril) for b in range(B)):
        return _numpy_ref(x, mask, position, qp, kvp, op, qns, kns)

    if "nc" not in _CACHE:
        _CACHE["nc"] = _build()
    nc = _CACHE["nc"]

    halves = []
    for half in range(2):
        halves.append((
            _pack(qp[half * NH:(half + 1) * NH]),
            _pack(kvp[0, half * NKV:(half + 1) * NKV]),
            _pack(kvp[1, half * NKV:(half + 1) * NKV]),
            np.ascontiguousarray(
                op[half * NH:(half + 1) * NH].transpose(1, 0, 2).reshape(128, NH * D)
            ).astype(npbf16),
        ))
    qsc = ((1.0 + qns) / np.sqrt(H)).reshape(128, 1).astype(np.float32)
    ksc = ((1.0 + kns) * np.sqrt(H)).reshape(128, 1).astype(np.float32)
    ts = THETA ** (np.arange(64, dtype=np.float64) * 2.0 / H)
    pidx = np.arange(128)[:, None]
    fidx = np.arange(128)[None, :]
    cm = (fidx >= pidx).astype(npbf16)

    in_maps = []
    for c in range(8):
        b, half = c // 2, c % 2
        wq, wk, wv, wo = halves[half]
        ang = position[b].astype(np.float64)[None, :] / ts[:, None]
        sn = np.sin(ang).astype(np.float32)
        cs = np.cos(ang).astype(np.float32)
        in_maps.append({
            "xt": np.ascontiguousarray(x[b].T).astype(npbf16),
            "wq": wq, "wk": wk, "wv": wv, "wo": wo,
            "cs": np.ascontiguousarray(np.concatenate([cs, cs], 0)).astype(npbf16),
            "sn": np.ascontiguousarray(np.concatenate([-sn, sn], 0)).astype(npbf16),
            "qsc": qsc, "ksc": ksc, "cm": cm,
        })

    if TRACE:
        _install_hook()
    last_err = None
    for _ in range(3):
        try:
            res = bass_utils.run_bass_kernel_spmd(nc, in_maps, list(range(8)), trace=TRACE)
            break
        except Exception as e:  # transient NRT device wedge
            last_err = e
    else:
        raise last_err
    LAST_EXEC_NS = getattr(res, "exec_time_ns", None)

    out = np.empty((B, T, D), np.float32)
    for b in range(B):
        out[b] = (res.results[2 * b]["out"].astype(np.float32)
                  + res.results[2 * b + 1]["out"].astype(np.float32))
    return out


# revision 23
# speedup vs baseline: 1.4118x; 1.0572x over previous
"""MultiHeadAttention (B=4,T=2048,D=2048,NQ=16,NK=8,H=128) on 8 trn2 cores.

Sharding: core c -> batch b=c//2, half=c%2. Each core computes the partial
output for batch b restricted to q-heads [half*8, half*8+8) (kv-heads
[half*4, half*4+4)); host sums the two partials per batch (o_proj
contraction over heads is split across the core pair).

v2: bf16 matmul/vector datapath (PSUM accumulation stays f32), fused
q/k/v projection pass with SBUF-resident q (no DRAM spill), DMA-engine
transposes for V, batched RMSNorm row statistics, f32r broadcast
matmuls, causal-restricted score/exp/PV tiles, software-pipelined
exp/PV attention loop with o_proj fused per 512-row query chunk.
"""
import numpy as np
import concourse.bass as bass
import concourse.tile as tile
from concourse import bacc, mybir
from concourse import bass_utils

B, T, D = 4, 2048, 2048
NQ, NK, H = 16, 8, 128
NH, NKV = 8, 4          # per-core q heads / kv heads
THETA = 10000.0
EPS = 1e-6
TCH = 512               # chunk of T for projections / attention q blocks
NCH = T // TCH
NDK = D // 128
NQC = T // TCH

f32 = mybir.dt.float32
f32r = mybir.dt.float32r
bf16 = mybir.dt.bfloat16
npbf16 = mybir.dt.np(bf16)
AF = mybir.ActivationFunctionType
MUL = mybir.AluOpType.mult

TRACE = False
LAST_EXEC_NS = None
_CACHE = {}


def _install_hook():
    import contextlib, ctypes, sys, types
    if "antenv.axon_hooks" in sys.modules:
        return
    lib = ctypes.CDLL("/opt/axon/libaxon_pjrt.so")
    lib.axon_start_nrt_profile.argtypes = [ctypes.POINTER(ctypes.c_int64), ctypes.c_size_t]
    lib.axon_start_nrt_profile.restype = ctypes.c_int64
    lib.axon_stop_nrt_profile.argtypes = [ctypes.c_char_p]
    lib.axon_stop_nrt_profile.restype = ctypes.c_int64

    @contextlib.contextmanager
    def _hook(output_dir, device_ids):
        import jax
        jax.devices()
        ids = (ctypes.c_int64 * len(device_ids))(*device_ids) if device_ids else None
        rc = lib.axon_start_nrt_profile(ids, len(device_ids) if device_ids else 0)
        if rc != 0:
            raise RuntimeError(f"axon_start_nrt_profile rc={rc}")
        try:
            yield
        finally:
            n = lib.axon_stop_nrt_profile(str(output_dir).encode())
            if n < 0:
                raise RuntimeError(f"axon_stop_nrt_profile rc={n}")

    mod = types.ModuleType("antenv.axon_hooks")
    mod.get_axon_ntff_profile_hook = lambda: _hook
    mod.set_axon_ntff_profile_hook = lambda h: None
    sys.modules["antenv.axon_hooks"] = mod
    bass_utils.upload_artifacts = lambda tmpdir: "local://" + str(tmpdir)


def _build():
    nc = bacc.Bacc("TRN2", target_bir_lowering=False, debug=False, num_devices=8)
    xt_ap = nc.dram_tensor("xt", [D, T], bf16, kind="ExternalInput").ap()
    wq_ap = nc.dram_tensor("wq", [128, NH * NDK * 128], bf16, kind="ExternalInput").ap()
    wk_ap = nc.dram_tensor("wk", [128, NKV * NDK * 128], bf16, kind="ExternalInput").ap()
    wv_ap = nc.dram_tensor("wv", [128, NKV * NDK * 128], bf16, kind="ExternalInput").ap()
    wo_ap = nc.dram_tensor("wo", [128, NH * D], bf16, kind="ExternalInput").ap()
    cs_ap = nc.dram_tensor("cs", [128, T], bf16, kind="ExternalInput").ap()
    sn_ap = nc.dram_tensor("sn", [128, T], bf16, kind="ExternalInput").ap()
    qsc_ap = nc.dram_tensor("qsc", [128, 1], f32, kind="ExternalInput").ap()
    ksc_ap = nc.dram_tensor("ksc", [128, 1], f32, kind="ExternalInput").ap()
    cm_ap = nc.dram_tensor("cm", [128, 128], bf16, kind="ExternalInput").ap()
    rm_ap = nc.dram_tensor("rmat", [128, 128], bf16, kind="ExternalInput").ap()
    out_ap = nc.dram_tensor("out", [T, D], bf16, kind="ExternalOutput").ap()

    with tile.TileContext(nc) as tc:
        with tc.tile_pool(name="mp", bufs=1) as mp, \
             tc.tile_pool(name="pp", bufs=1, space="PSUM") as pp:
            # ---- persistent tiles ----
            cs_t = mp.tile([128, T], bf16)
            nc.sync.dma_start(cs_t[:], cs_ap[:])
            sn_t = mp.tile([128, T], bf16)
            nc.sync.dma_start(sn_t[:], sn_ap[:])
            qsc_t = mp.tile([128, 1], f32)
            nc.sync.dma_start(qsc_t[:], qsc_ap[:])
            ksc_t = mp.tile([128, 1], f32)
            nc.sync.dma_start(ksc_t[:], ksc_ap[:])
            cm_t = mp.tile([128, 128], bf16)
            nc.sync.dma_start(cm_t[:], cm_ap[:])
            rmat_t = mp.tile([128, 128], bf16)
            nc.sync.dma_start(rmat_t[:], rm_ap[:])
            eps_t = mp.tile([1, 1], f32)
            nc.vector.memset(eps_t[:], EPS)
            ones_col_b = mp.tile([128, 1], bf16)
            nc.vector.memset(ones_col_b[:], 1.0)
            ones_row_b = mp.tile([1, 128], bf16)
            nc.vector.memset(ones_row_b[:], 1.0)

            qT = mp.tile([128, NH * T], bf16)     # 32KB/part
            kT = mp.tile([128, NKV * T], bf16)    # 16KB
            vT = mp.tile([128, NKV * T], bf16)    # 16KB

            # PSUM bank for softmax denominator rows (attention phase).
            rows = pp.tile([128, 512], f32, tag="rows")

            def drain_group(pool, accs, sc_t, t0, dsts):
                """accs: psum [128,512] f32 -> RMSNorm*(gain) + RoPE -> dsts bf16."""
                n = len(accs)
                for i in range(n):
                    sq = pool.tile([128, TCH], bf16, tag="sq", bufs=2, name=f"sq{i}")
                    nc.scalar.activation(sq[:], accs[i], AF.Square)
                    row = pp.tile([1, 512], f32, tag=f"a{3 + i % 2}", name=f"row{i}")
                    nc.tensor.matmul(row[:], ones_col_b[:], sq[:], start=True, stop=True)
                    rinv = pool.tile([1, TCH], f32, tag="rinv", bufs=2, name=f"rinv{i}")
                    nc.vector.reciprocal_approx_fast(rinv[:], row[:])
                    rstd = pool.tile([1, TCH], bf16, tag="rstd", bufs=2, name=f"rstd{i}")
                    nc.scalar.activation(rstd[:], rinv[:], AF.Sqrt)
                    bc = pp.tile([128, 512], f32, tag=f"a{3 + (i + 1) % 2}", name=f"bc{i}")
                    nc.tensor.matmul(bc[:], ones_row_b[:], rstd[:],
                                     start=True, stop=True)
                    # qn0 = acc * gain (per-partition); rstd applied after rope
                    # (valid: rstd is per-column, invariant under the half-swap)
                    qn = pool.tile([128, TCH], bf16, tag="qn", bufs=2, name=f"qn{i}")
                    nc.scalar.activation(qn[:], accs[i], AF.Copy, scale=sc_t[:])
                    qsw = pp.tile([128, 512], f32, tag=f"a{5 + i % 2}", name=f"qsw{i}")
                    nc.tensor.matmul(qsw[:], rmat_t[:], qn[:], start=True, stop=True)
                    ta = pool.tile([128, TCH], bf16, tag="ta", bufs=2, name=f"ta{i}")
                    nc.vector.tensor_mul(ta[:], qn[:], cs_t[:, t0:t0 + TCH])
                    tb = pool.tile([128, TCH], bf16, tag="tb", bufs=2, name=f"tb{i}")
                    nc.vector.tensor_mul(tb[:], qsw[:], sn_t[:, t0:t0 + TCH])
                    rs = pool.tile([128, TCH], bf16, tag="rs", bufs=2, name=f"rs{i}")
                    nc.vector.tensor_add(rs[:], ta[:], tb[:])
                    nc.vector.tensor_mul(dsts[i], rs[:], bc[:])

            # ---- phase 1: q/k/v projection, norm+rope, all SBUF-resident ----
            with tc.tile_pool(name="pj", bufs=1) as pj:
                wq_t = pj.tile([128, NH * NDK * 128], bf16)
                for j in range(4):
                    s = NH * NDK * 128 // 4
                    nc.sync.dma_start(wq_t[:, j * s:(j + 1) * s], wq_ap[:, j * s:(j + 1) * s])
                wk_t = pj.tile([128, NKV * NDK * 128], bf16)
                for j in range(2):
                    s = NKV * NDK * 128 // 2
                    nc.sync.dma_start(wk_t[:, j * s:(j + 1) * s], wk_ap[:, j * s:(j + 1) * s])
                wv_t = pj.tile([128, NKV * NDK * 128], bf16)
                for j in range(2):
                    s = NKV * NDK * 128 // 2
                    nc.sync.dma_start(wv_t[:, j * s:(j + 1) * s], wv_ap[:, j * s:(j + 1) * s])

                acc_roll = [0]

                def chain(w_t, head, xh0, xh1):
                    """Sequential 16-dk matmul chain into one rolling psum bank."""
                    acc = pp.tile([128, 512], f32, tag=f"a{acc_roll[0] % 3}", name="acc")
                    acc_roll[0] += 1
                    for dk in range(NDK):
                        xh = xh0 if dk < 8 else xh1
                        nc.tensor.matmul(
                            acc[:],
                            w_t[:, (head * NDK + dk) * 128:(head * NDK + dk + 1) * 128],
                            xh[:, (dk % 8) * TCH:(dk % 8 + 1) * TCH],
                            start=(dk == 0), stop=(dk == NDK - 1))
                    return acc

                xt_r = xt_ap.rearrange("(a p) t -> p a t", p=128)

                def load_xh(ch):
                    t0 = ch * TCH
                    xh0 = pj.tile([128, 8 * TCH], bf16, tag="xh0", bufs=2, name="xh0")
                    nc.sync.dma_start(
                        xh0[:].rearrange("p (a t) -> p a t", a=8),
                        xt_r[:, 0:8, t0:t0 + TCH])
                    xh1 = pj.tile([128, 8 * TCH], bf16, tag="xh1", bufs=2, name="xh1")
                    nc.sync.dma_start(
                        xh1[:].rearrange("p (a t) -> p a t", a=8),
                        xt_r[:, 8:16, t0:t0 + TCH])
                    return xh0, xh1

                xhs = load_xh(0)
                for ch in range(NCH):
                    t0 = ch * TCH
                    xh0, xh1 = xhs
                    # prefetch next chunk's x before this chunk's drains/V
                    # DMAs enter the in-order SP queue
                    if ch + 1 < NCH:
                        xhs = load_xh(ch + 1)
                    # q heads, two groups of 4
                    for g in range(2):
                        accs = [chain(wq_t, g * 4 + i, xh0, xh1) for i in range(4)]
                        drain_group(pj, [a[:] for a in accs], qsc_t, t0,
                                    [qT[:, (g * 4 + i) * T + t0:(g * 4 + i) * T + t0 + TCH]
                                     for i in range(4)])
                    # k heads
                    accs = [chain(wk_t, i, xh0, xh1) for i in range(NKV)]
                    drain_group(pj, [a[:] for a in accs], ksc_t, t0,
                                [kT[:, kv * T + t0:kv * T + t0 + TCH] for kv in range(NKV)])
                    # v heads: no norm/rope; transpose into [t,H] blocks via DMA xbar
                    for kv in range(NKV):
                        acc = chain(wv_t, kv, xh0, xh1)
                        vtmp = pj.tile([128, TCH], bf16, tag="vtmp", bufs=2, name="vtmp")
                        nc.scalar.activation(vtmp[:], acc[:], AF.Copy)
                        for j in range(TCH // 128):
                            nc.sync.dma_start(
                                vT[:, kv * T + t0 + j * 128:kv * T + t0 + (j + 1) * 128],
                                vtmp[:, j * 128:(j + 1) * 128], transpose=True)

            # ---- phase 2: attention + fused o_proj ----
            with tc.tile_pool(name="op", bufs=1) as op:
                wo_t = op.tile([128, NH * D], bf16)
                for j in range(4):
                    s = NH * D // 4
                    nc.sync.dma_start(wo_t[:, j * s:(j + 1) * s], wo_ap[:, j * s:(j + 1) * s])

                for qi in range(NQC):
                    q0 = qi * TCH
                    attn_sb = op.tile([128, NH * TCH], bf16, tag="attn", bufs=2, name="attn")
                    nkj = 4 * qi + 4
                    for h in range(NH):
                        kv = h // 2
                        o_ps = pp.tile([128, 512], f32, tag=f"a{5 + h % 2}", name="ops")
                        acc_sb = op.tile([128, TCH], bf16, tag="accsb", bufs=2, name="accsb")

                        def emit_s(kj):
                            m = kj - 4 * qi
                            lo = 128 * m if m > 0 else 0
                            s_ps = pp.tile([128, 512], f32, tag=f"a{kj % 5}", name="sps")
                            nc.tensor.matmul(
                                s_ps[:, lo:512],
                                kT[:, kv * T + kj * 128:kv * T + (kj + 1) * 128],
                                qT[:, h * T + q0 + lo:h * T + q0 + TCH],
                                start=True, stop=True)
                            pt = op.tile([128, TCH], bf16, tag="pt", bufs=3, name="pt")
                            return s_ps, pt, lo, m, kj

                        def emit_drain(s_ps, pt, lo, m, kj):
                            nc.scalar.activation(pt[:, lo:512], s_ps[:, lo:512], AF.Exp)
                            if m >= 0:
                                nc.vector.tensor_mul(pt[:, lo:lo + 128],
                                                     pt[:, lo:lo + 128], cm_t[:])
                            if kj == 0:
                                nc.vector.tensor_copy(acc_sb[:], pt[:])
                            else:
                                nc.vector.tensor_add(acc_sb[:, lo:512],
                                                     acc_sb[:, lo:512], pt[:, lo:512])
                            nc.tensor.matmul(
                                o_ps[:, lo:512],
                                vT[:, kv * T + kj * 128:kv * T + (kj + 1) * 128],
                                pt[:, lo:512],
                                start=(kj == 0), stop=(kj == nkj - 1),
                                skip_group_check=True)

                        prev = None
                        for kj in range(nkj):
                            cur = emit_s(kj)
                            if prev is not None:
                                emit_drain(*prev)
                            prev = cur
                        emit_drain(*prev)

                        # softmax denominator for this (h, qi).
                        # NB: reciprocal_approx_fast corrupts results when its
                        # input sits at a nonzero base partition -> keep row 0.
                        nc.tensor.matmul(rows[0:1, :], ones_col_b[:],
                                         acc_sb[:], start=True, stop=True)
                        rden = op.tile([1, TCH], f32, tag="rden", bufs=2, name="rden")
                        nc.vector.reciprocal_approx_fast(rden[:], rows[0:1, :])
                        rbc = op.tile([128, TCH], f32, tag="rbc", bufs=2, name="rbc")
                        nc.gpsimd.partition_broadcast(rbc[:], rden[:])
                        nc.vector.tensor_mul(
                            attn_sb[:, h * TCH:(h + 1) * TCH], o_ps[:], rbc[:])

                    # fused o_proj for this 512-row query chunk
                    out_r = out_ap.rearrange("(a p) d -> p a d", p=128)
                    for dc in range(4):
                        stg4 = op.tile([128, 4 * 512], bf16, tag="ostg", bufs=2, name="ostg")
                        for ti in range(4):
                            ops2 = pp.tile([128, 512], f32,
                                           tag=f"a{5 + (dc * 4 + ti) % 2}", name="ops2")
                            for h in range(NH):
                                nc.tensor.matmul(
                                    ops2[:],
                                    attn_sb[:, h * TCH + ti * 128:h * TCH + (ti + 1) * 128],
                                    wo_t[:, h * D + dc * 512:h * D + (dc + 1) * 512],
                                    start=(h == 0), stop=(h == NH - 1))
                            if (dc * 4 + ti) % 2 == 0:
                                nc.vector.tensor_copy(stg4[:, ti * 512:(ti + 1) * 512], ops2[:])
                            else:
                                nc.scalar.activation(stg4[:, ti * 512:(ti + 1) * 512],
                                                     ops2[:], AF.Copy)
                        nc.sync.dma_start(
                            out_r[:, qi * 4:qi * 4 + 4, dc * 512:(dc + 1) * 512],
                            stg4[:].rearrange("p (a d) -> p a d", a=4))

    nc.compile()
    return nc


def _pack(w):
    """(nh, D, H) -> (128, nh*NDK*128): col block (h*NDK+dk)*128 = w[h, dk*128:+128, :]."""
    nh = w.shape[0]
    a = w.reshape(nh, NDK, 128, H).transpose(2, 0, 1, 3)
    return np.ascontiguousarray(a.reshape(128, nh * NDK * H)).astype(npbf16)


def _numpy_ref(x, mask, position, qp, kvp, op, qns, kns):
    def rms(v, s):
        var = (v * v).mean(-1, keepdims=True)
        return v / np.sqrt(var + EPS) * (1.0 + s)

    def rope(v, pos):
        ts = THETA ** (np.arange(64, dtype=np.float32) * 2.0 / H)
        ang = pos.astype(np.float32)[:, :, None, None] / ts
        sn, cs = np.sin(ang), np.cos(ang)
        x1, x2 = v[..., :64], v[..., 64:]
        return np.concatenate([x1 * cs - x2 * sn, x2 * cs + x1 * sn], -1)

    q = np.einsum('BTD,NDH->BTNH', x, qp)
    k = np.einsum('BTD,KDH->BTKH', x, kvp[0])
    v = np.einsum('BTD,KDH->BTKH', x, kvp[1])
    q = rope(rms(q, qns), position) * (H ** -0.5)
    k = rope(rms(k, kns), position)
    q = q.transpose(0, 2, 1, 3)
    k = np.repeat(k.transpose(0, 2, 1, 3), NQ // NK, 1)
    v = np.repeat(v.transpose(0, 2, 1, 3), NQ // NK, 1)
    s = np.einsum('BHtD,BHTD->BHtT', q, k) / np.sqrt(np.float32(H))
    s = np.where(mask[:, None], s, np.float32(-2.3819763e+38))
    s = s - s.max(-1, keepdims=True)
    w = np.exp(s)
    w /= w.sum(-1, keepdims=True)
    o = np.einsum('BHtT,BHTD->BHtD', w, v)
    return np.einsum('BNTH,NHD->BTD', o, op).astype(np.float32)


def kernel(**inputs):
    global LAST_EXEC_NS
    x = np.asarray(inputs["x"], np.float32)
    mask = np.asarray(inputs["mask"])
    position = np.asarray(inputs["position"])
    qp = np.asarray(inputs["q_proj"], np.float32)
    kvp = np.asarray(inputs["kv_proj"], np.float32)
    op = np.asarray(inputs["o_proj"], np.float32)
    qns = np.asarray(inputs["q_norm_scale"], np.float32)
    kns = np.asarray(inputs["k_norm_scale"], np.float32)

    tril = np.tril(np.ones((T, T), bool))
    if mask.shape != (B, T, T) or not all(np.array_equal(mask[b], tril) for b in range(B)):
        return _numpy_ref(x, mask, position, qp, kvp, op, qns, kns)

    if "nc" not in _CACHE:
        _CACHE["nc"] = _build()
    nc = _CACHE["nc"]

    halves = []
    for half in range(2):
        halves.append((
            _pack(qp[half * NH:(half + 1) * NH]),
            _pack(kvp[0, half * NKV:(half + 1) * NKV]),
            _pack(kvp[1, half * NKV:(half + 1) * NKV]),
            np.ascontiguousarray(
                op[half * NH:(half + 1) * NH].transpose(1, 0, 2).reshape(128, NH * D)
            ).astype(npbf16),
        ))
    qsc = ((1.0 + qns) / np.sqrt(H)).reshape(128, 1).astype(np.float32)
    ksc = ((1.0 + kns) * np.sqrt(H)).reshape(128, 1).astype(np.float32)
    ts = THETA ** (np.arange(64, dtype=np.float64) * 2.0 / H)
    pidx = np.arange(128)[:, None]
    fidx = np.arange(128)[None, :]
    cm = (fidx >= pidx).astype(npbf16)
    rmat = np.zeros((128, 128), np.float32)
    rmat[(np.arange(128) + 64) % 128, np.arange(128)] = 1.0
    rmat = rmat.astype(npbf16)

    in_maps = []
    for c in range(8):
        b, half = c // 2, c % 2
        wq, wk, wv, wo = halves[half]
        ang = position[b].astype(np.float64)[None, :] / ts[:, None]
        sn = np.sin(ang).astype(np.float32)
        cs = np.cos(ang).astype(np.float32)
        in_maps.append({
            "xt": np.ascontiguousarray(x[b].T).astype(npbf16),
            "wq": wq, "wk": wk, "wv": wv, "wo": wo,
            "cs": np.ascontiguousarray(np.concatenate([cs, cs], 0)).astype(npbf16),
            "sn": np.ascontiguousarray(np.concatenate([-sn, sn], 0)).astype(npbf16),
            "qsc": qsc, "ksc": ksc, "cm": cm, "rmat": rmat,
        })

    if TRACE:
        _install_hook()
    last_err = None
    for _ in range(3):
        try:
            res = bass_utils.run_bass_kernel_spmd(nc, in_maps, list(range(8)), trace=TRACE)
            break
        except Exception as e:  # transient NRT device wedge
            last_err = e
    else:
        raise last_err
    LAST_EXEC_NS = getattr(res, "exec_time_ns", None)

    out = np.empty((B, T, D), np.float32)
    for b in range(B):
        out[b] = (res.results[2 * b]["out"].astype(np.float32)
                  + res.results[2 * b + 1]["out"].astype(np.float32))
    return out


# revision 24
# speedup vs baseline: 1.4278x; 1.0113x over previous
"""MultiHeadAttention (B=4,T=2048,D=2048,NQ=16,NK=8,H=128) on 8 trn2 cores.

Sharding: core c -> batch b=c//2, half=c%2. Each core computes the partial
output for batch b restricted to q-heads [half*8, half*8+8) (kv-heads
[half*4, half*4+4)); host sums the two partials per batch (o_proj
contraction over heads is split across the core pair).

v2: bf16 matmul/vector datapath (PSUM accumulation stays f32), fused
q/k/v projection pass with SBUF-resident q (no DRAM spill), DMA-engine
transposes for V, batched RMSNorm row statistics, f32r broadcast
matmuls, causal-restricted score/exp/PV tiles, software-pipelined
exp/PV attention loop with o_proj fused per 512-row query chunk.
"""
import numpy as np
import concourse.bass as bass
import concourse.tile as tile
from concourse import bacc, mybir
from concourse import bass_utils

B, T, D = 4, 2048, 2048
NQ, NK, H = 16, 8, 128
NH, NKV = 8, 4          # per-core q heads / kv heads
THETA = 10000.0
EPS = 1e-6
TCH = 512               # chunk of T for projections / attention q blocks
NCH = T // TCH
NDK = D // 128
NQC = T // TCH

f32 = mybir.dt.float32
f32r = mybir.dt.float32r
bf16 = mybir.dt.bfloat16
npbf16 = mybir.dt.np(bf16)
AF = mybir.ActivationFunctionType
MUL = mybir.AluOpType.mult

TRACE = False
LAST_EXEC_NS = None
_CACHE = {}


def _install_hook():
    import contextlib, ctypes, sys, types
    if "antenv.axon_hooks" in sys.modules:
        return
    lib = ctypes.CDLL("/opt/axon/libaxon_pjrt.so")
    lib.axon_start_nrt_profile.argtypes = [ctypes.POINTER(ctypes.c_int64), ctypes.c_size_t]
    lib.axon_start_nrt_profile.restype = ctypes.c_int64
    lib.axon_stop_nrt_profile.argtypes = [ctypes.c_char_p]
    lib.axon_stop_nrt_profile.restype = ctypes.c_int64

    @contextlib.contextmanager
    def _hook(output_dir, device_ids):
        import jax
        jax.devices()
        ids = (ctypes.c_int64 * len(device_ids))(*device_ids) if device_ids else None
        rc = lib.axon_start_nrt_profile(ids, len(device_ids) if device_ids else 0)
        if rc != 0:
            raise RuntimeError(f"axon_start_nrt_profile rc={rc}")
        try:
            yield
        finally:
            n = lib.axon_stop_nrt_profile(str(output_dir).encode())
            if n < 0:
                raise RuntimeError(f"axon_stop_nrt_profile rc={n}")

    mod = types.ModuleType("antenv.axon_hooks")
    mod.get_axon_ntff_profile_hook = lambda: _hook
    mod.set_axon_ntff_profile_hook = lambda h: None
    sys.modules["antenv.axon_hooks"] = mod
    bass_utils.upload_artifacts = lambda tmpdir: "local://" + str(tmpdir)


def _build():
    nc = bacc.Bacc("TRN2", target_bir_lowering=False, debug=False, num_devices=8)
    xt_ap = nc.dram_tensor("xt", [D, T], bf16, kind="ExternalInput").ap()
    wq_ap = nc.dram_tensor("wq", [128, NH * NDK * 128], bf16, kind="ExternalInput").ap()
    wk_ap = nc.dram_tensor("wk", [128, NKV * NDK * 128], bf16, kind="ExternalInput").ap()
    wv_ap = nc.dram_tensor("wv", [128, NKV * NDK * 128], bf16, kind="ExternalInput").ap()
    wo_ap = nc.dram_tensor("wo", [128, NH * D], bf16, kind="ExternalInput").ap()
    cs_ap = nc.dram_tensor("cs", [128, T], bf16, kind="ExternalInput").ap()
    sn_ap = nc.dram_tensor("sn", [128, T], bf16, kind="ExternalInput").ap()
    qsc_ap = nc.dram_tensor("qsc", [128, 1], f32, kind="ExternalInput").ap()
    ksc_ap = nc.dram_tensor("ksc", [128, 1], f32, kind="ExternalInput").ap()
    cm_ap = nc.dram_tensor("cm", [128, 128], bf16, kind="ExternalInput").ap()
    rm_ap = nc.dram_tensor("rmat", [128, 128], bf16, kind="ExternalInput").ap()
    out_ap = nc.dram_tensor("out", [T, D], bf16, kind="ExternalOutput").ap()

    with tile.TileContext(nc) as tc:
        with tc.tile_pool(name="mp", bufs=1) as mp, \
             tc.tile_pool(name="pp", bufs=1, space="PSUM") as pp:
            # ---- persistent tiles ----
            cs_t = mp.tile([128, T], bf16)
            nc.sync.dma_start(cs_t[:], cs_ap[:])
            sn_t = mp.tile([128, T], bf16)
            nc.sync.dma_start(sn_t[:], sn_ap[:])
            qsc_t = mp.tile([128, 1], f32)
            nc.sync.dma_start(qsc_t[:], qsc_ap[:])
            ksc_t = mp.tile([128, 1], f32)
            nc.sync.dma_start(ksc_t[:], ksc_ap[:])
            cm_t = mp.tile([128, 128], bf16)
            nc.sync.dma_start(cm_t[:], cm_ap[:])
            rmat_t = mp.tile([128, 128], bf16)
            nc.sync.dma_start(rmat_t[:], rm_ap[:])
            eps_t = mp.tile([1, 1], f32)
            nc.vector.memset(eps_t[:], EPS)
            ones_col_b = mp.tile([128, 1], bf16)
            nc.vector.memset(ones_col_b[:], 1.0)
            ones_row_b = mp.tile([1, 128], bf16)
            nc.vector.memset(ones_row_b[:], 1.0)

            qT = mp.tile([128, NH * T], bf16)     # 32KB/part
            kT = mp.tile([128, NKV * T], bf16)    # 16KB
            vT = mp.tile([128, NKV * T], bf16)    # 16KB

            # PSUM bank for softmax denominator rows (attention phase).
            rows = pp.tile([128, 512], f32, tag="rows")

            def drain_group(pool, accs, sc_t, t0, dsts):
                """accs: psum [128,512] f32 -> RMSNorm*(gain) + RoPE -> dsts bf16."""
                n = len(accs)
                for i in range(n):
                    sq = pool.tile([128, TCH], bf16, tag="sq", bufs=2, name=f"sq{i}")
                    nc.scalar.activation(sq[:], accs[i], AF.Square)
                    row = pp.tile([1, 512], f32, tag=f"a{3 + i % 2}", name=f"row{i}")
                    nc.tensor.matmul(row[:], ones_col_b[:], sq[:], start=True, stop=True)
                    rinv = pool.tile([1, TCH], f32, tag="rinv", bufs=2, name=f"rinv{i}")
                    nc.vector.reciprocal_approx_fast(rinv[:], row[:])
                    rstd = pool.tile([1, TCH], bf16, tag="rstd", bufs=2, name=f"rstd{i}")
                    nc.scalar.activation(rstd[:], rinv[:], AF.Sqrt)
                    bc = pp.tile([128, 512], f32, tag=f"a{3 + (i + 1) % 2}", name=f"bc{i}")
                    nc.tensor.matmul(bc[:], ones_row_b[:], rstd[:],
                                     start=True, stop=True)
                    # qn0 = acc * gain (per-partition); rstd applied after rope
                    # (valid: rstd is per-column, invariant under the half-swap)
                    qn = pool.tile([128, TCH], bf16, tag="qn", bufs=2, name=f"qn{i}")
                    nc.scalar.activation(qn[:], accs[i], AF.Copy, scale=sc_t[:])
                    qsw = pp.tile([128, 512], f32, tag=f"a{5 + i % 2}", name=f"qsw{i}")
                    nc.tensor.matmul(qsw[:], rmat_t[:], qn[:], start=True, stop=True)
                    ta = pool.tile([128, TCH], bf16, tag="ta", bufs=2, name=f"ta{i}")
                    nc.vector.tensor_mul(ta[:], qn[:], cs_t[:, t0:t0 + TCH])
                    tb = pool.tile([128, TCH], bf16, tag="tb", bufs=2, name=f"tb{i}")
                    nc.vector.tensor_mul(tb[:], qsw[:], sn_t[:, t0:t0 + TCH])
                    rs = pool.tile([128, TCH], bf16, tag="rs", bufs=2, name=f"rs{i}")
                    nc.vector.tensor_add(rs[:], ta[:], tb[:])
                    nc.vector.tensor_mul(dsts[i], rs[:], bc[:])

            # ---- phase 1: q/k/v projection, norm+rope, all SBUF-resident ----
            with tc.tile_pool(name="pj", bufs=1) as pj:
                wq_t = pj.tile([128, NH * NDK * 128], bf16)
                for j in range(4):
                    s = NH * NDK * 128 // 4
                    nc.sync.dma_start(wq_t[:, j * s:(j + 1) * s], wq_ap[:, j * s:(j + 1) * s])
                wk_t = pj.tile([128, NKV * NDK * 128], bf16)
                for j in range(2):
                    s = NKV * NDK * 128 // 2
                    nc.sync.dma_start(wk_t[:, j * s:(j + 1) * s], wk_ap[:, j * s:(j + 1) * s])
                wv_t = pj.tile([128, NKV * NDK * 128], bf16)
                for j in range(2):
                    s = NKV * NDK * 128 // 2
                    nc.sync.dma_start(wv_t[:, j * s:(j + 1) * s], wv_ap[:, j * s:(j + 1) * s])

                acc_roll = [0]

                def chain(w_t, head, xh0, xh1):
                    """Sequential 16-dk matmul chain into one rolling psum bank."""
                    acc = pp.tile([128, 512], f32, tag=f"a{acc_roll[0] % 3}", name="acc")
                    acc_roll[0] += 1
                    for dk in range(NDK):
                        xh = xh0 if dk < 8 else xh1
                        nc.tensor.matmul(
                            acc[:],
                            w_t[:, (head * NDK + dk) * 128:(head * NDK + dk + 1) * 128],
                            xh[:, (dk % 8) * TCH:(dk % 8 + 1) * TCH],
                            start=(dk == 0), stop=(dk == NDK - 1))
                    return acc

                xt_r = xt_ap.rearrange("(a p) t -> p a t", p=128)

                def load_xh(ch):
                    t0 = ch * TCH
                    xh0 = pj.tile([128, 8 * TCH], bf16, tag="xh0", bufs=2, name="xh0")
                    nc.sync.dma_start(
                        xh0[:].rearrange("p (a t) -> p a t", a=8),
                        xt_r[:, 0:8, t0:t0 + TCH])
                    xh1 = pj.tile([128, 8 * TCH], bf16, tag="xh1", bufs=2, name="xh1")
                    nc.sync.dma_start(
                        xh1[:].rearrange("p (a t) -> p a t", a=8),
                        xt_r[:, 8:16, t0:t0 + TCH])
                    return xh0, xh1

                xhs = load_xh(0)
                for ch in range(NCH):
                    t0 = ch * TCH
                    xh0, xh1 = xhs
                    # prefetch next chunk's x before this chunk's drains/V
                    # DMAs enter the in-order SP queue
                    if ch + 1 < NCH:
                        xhs = load_xh(ch + 1)
                    # q heads, two groups of 4
                    for g in range(2):
                        accs = [chain(wq_t, g * 4 + i, xh0, xh1) for i in range(4)]
                        drain_group(pj, [a[:] for a in accs], qsc_t, t0,
                                    [qT[:, (g * 4 + i) * T + t0:(g * 4 + i) * T + t0 + TCH]
                                     for i in range(4)])
                    # k heads
                    accs = [chain(wk_t, i, xh0, xh1) for i in range(NKV)]
                    drain_group(pj, [a[:] for a in accs], ksc_t, t0,
                                [kT[:, kv * T + t0:kv * T + t0 + TCH] for kv in range(NKV)])
                    # v heads: computed directly in [t, H] orientation
                    # (lhsT = x block, rhs = w column), so no transpose needed
                    for kv in range(NKV):
                        vps = pp.tile([128, 512], f32, tag=f"a{acc_roll[0] % 3}", name="vps")
                        acc_roll[0] += 1
                        for tb_ in range(4):
                            for dk in range(NDK):
                                xh = xh0 if dk < 8 else xh1
                                nc.tensor.matmul(
                                    vps[:, tb_ * 128:(tb_ + 1) * 128],
                                    xh[:, (dk % 8) * TCH + tb_ * 128:(dk % 8) * TCH + (tb_ + 1) * 128],
                                    wv_t[:, (kv * NDK + dk) * 128:(kv * NDK + dk + 1) * 128],
                                    start=(dk == 0), stop=(dk == NDK - 1),
                                    skip_group_check=True)
                        nc.vector.tensor_copy(vT[:, kv * T + t0:kv * T + t0 + TCH], vps[:])

            # ---- phase 2: attention + fused o_proj ----
            with tc.tile_pool(name="op", bufs=1) as op:
                wo_t = op.tile([128, NH * D], bf16)
                for j in range(4):
                    s = NH * D // 4
                    nc.sync.dma_start(wo_t[:, j * s:(j + 1) * s], wo_ap[:, j * s:(j + 1) * s])

                for qi in range(NQC):
                    q0 = qi * TCH
                    attn_sb = op.tile([128, NH * TCH], bf16, tag="attn", bufs=2, name="attn")
                    nkj = 4 * qi + 4
                    for h in range(NH):
                        kv = h // 2
                        o_ps = pp.tile([128, 512], f32, tag=f"a{5 + h % 2}", name="ops")
                        acc_sb = op.tile([128, TCH], bf16, tag="accsb", bufs=2, name="accsb")

                        def emit_s(kj):
                            m = kj - 4 * qi
                            lo = 128 * m if m > 0 else 0
                            s_ps = pp.tile([128, 512], f32, tag=f"a{kj % 5}", name="sps")
                            nc.tensor.matmul(
                                s_ps[:, lo:512],
                                kT[:, kv * T + kj * 128:kv * T + (kj + 1) * 128],
                                qT[:, h * T + q0 + lo:h * T + q0 + TCH],
                                start=True, stop=True)
                            pt = op.tile([128, TCH], bf16, tag="pt", bufs=3, name="pt")
                            return s_ps, pt, lo, m, kj

                        def emit_drain(s_ps, pt, lo, m, kj):
                            nc.scalar.activation(pt[:, lo:512], s_ps[:, lo:512], AF.Exp)
                            if m >= 0:
                                nc.vector.tensor_mul(pt[:, lo:lo + 128],
                                                     pt[:, lo:lo + 128], cm_t[:])
                            if kj == 0:
                                nc.vector.tensor_copy(acc_sb[:], pt[:])
                            else:
                                nc.vector.tensor_add(acc_sb[:, lo:512],
                                                     acc_sb[:, lo:512], pt[:, lo:512])
                            nc.tensor.matmul(
                                o_ps[:, lo:512],
                                vT[:, kv * T + kj * 128:kv * T + (kj + 1) * 128],
                                pt[:, lo:512],
                                start=(kj == 0), stop=(kj == nkj - 1),
                                skip_group_check=True)

                        prev = None
                        for kj in range(nkj):
                            cur = emit_s(kj)
                            if prev is not None:
                                emit_drain(*prev)
                            prev = cur
                        emit_drain(*prev)

                        # softmax denominator for this (h, qi).
                        # NB: reciprocal_approx_fast corrupts results when its
                        # input sits at a nonzero base partition -> keep row 0.
                        nc.tensor.matmul(rows[0:1, :], ones_col_b[:],
                                         acc_sb[:], start=True, stop=True)
                        rden = op.tile([1, TCH], f32, tag="rden", bufs=2, name="rden")
                        nc.vector.reciprocal_approx_fast(rden[:], rows[0:1, :])
                        rbc = op.tile([128, TCH], f32, tag="rbc", bufs=2, name="rbc")
                        nc.gpsimd.partition_broadcast(rbc[:], rden[:])
                        nc.vector.tensor_mul(
                            attn_sb[:, h * TCH:(h + 1) * TCH], o_ps[:], rbc[:])

                    # fused o_proj for this 512-row query chunk
                    out_r = out_ap.rearrange("(a p) d -> p a d", p=128)
                    for dc in range(4):
                        stg4 = op.tile([128, 4 * 512], bf16, tag="ostg", bufs=2, name="ostg")
                        for ti in range(4):
                            ops2 = pp.tile([128, 512], f32,
                                           tag=f"a{5 + (dc * 4 + ti) % 2}", name="ops2")
                            for h in range(NH):
                                nc.tensor.matmul(
                                    ops2[:],
                                    attn_sb[:, h * TCH + ti * 128:h * TCH + (ti + 1) * 128],
                                    wo_t[:, h * D + dc * 512:h * D + (dc + 1) * 512],
                                    start=(h == 0), stop=(h == NH - 1))
                            if (dc * 4 + ti) % 2 == 0:
                                nc.vector.tensor_copy(stg4[:, ti * 512:(ti + 1) * 512], ops2[:])
                            else:
                                nc.scalar.activation(stg4[:, ti * 512:(ti + 1) * 512],
                                                     ops2[:], AF.Copy)
                        nc.sync.dma_start(
                            out_r[:, qi * 4:qi * 4 + 4, dc * 512:(dc + 1) * 512],
                            stg4[:].rearrange("p (a d) -> p a d", a=4))

    nc.compile()
    return nc


def _pack(w):
    """(nh, D, H) -> (128, nh*NDK*128): col block (h*NDK+dk)*128 = w[h, dk*128:+128, :]."""
    nh = w.shape[0]
    a = w.reshape(nh, NDK, 128, H).transpose(2, 0, 1, 3)
    return np.ascontiguousarray(a.reshape(128, nh * NDK * H)).astype(npbf16)


def _numpy_ref(x, mask, position, qp, kvp, op, qns, kns):
    def rms(v, s):
        var = (v * v).mean(-1, keepdims=True)
        return v / np.sqrt(var + EPS) * (1.0 + s)

    def rope(v, pos):
        ts = THETA ** (np.arange(64, dtype=np.float32) * 2.0 / H)
        ang = pos.astype(np.float32)[:, :, None, None] / ts
        sn, cs = np.sin(ang), np.cos(ang)
        x1, x2 = v[..., :64], v[..., 64:]
        return np.concatenate([x1 * cs - x2 * sn, x2 * cs + x1 * sn], -1)

    q = np.einsum('BTD,NDH->BTNH', x, qp)
    k = np.einsum('BTD,KDH->BTKH', x, kvp[0])
    v = np.einsum('BTD,KDH->BTKH', x, kvp[1])
    q = rope(rms(q, qns), position) * (H ** -0.5)
    k = rope(rms(k, kns), position)
    q = q.transpose(0, 2, 1, 3)
    k = np.repeat(k.transpose(0, 2, 1, 3), NQ // NK, 1)
    v = np.repeat(v.transpose(0, 2, 1, 3), NQ // NK, 1)
    s = np.einsum('BHtD,BHTD->BHtT', q, k) / np.sqrt(np.float32(H))
    s = np.where(mask[:, None], s, np.float32(-2.3819763e+38))
    s = s - s.max(-1, keepdims=True)
    w = np.exp(s)
    w /= w.sum(-1, keepdims=True)
    o = np.einsum('BHtT,BHTD->BHtD', w, v)
    return np.einsum('BNTH,NHD->BTD', o, op).astype(np.float32)


def kernel(**inputs):
    global LAST_EXEC_NS
    x = np.asarray(inputs["x"], np.float32)
    mask = np.asarray(inputs["mask"])
    position = np.asarray(inputs["position"])
    qp = np.asarray(inputs["q_proj"], np.float32)
    kvp = np.asarray(inputs["kv_proj"], np.float32)
    op = np.asarray(inputs["o_proj"], np.float32)
    qns = np.asarray(inputs["q_norm_scale"], np.float32)
    kns = np.asarray(inputs["k_norm_scale"], np.float32)

    tril = np.tril(np.ones((T, T), bool))
    if mask.shape != (B, T, T) or not all(np.array_equal(mask[b], tril) for b in range(B)):
        return _numpy_ref(x, mask, position, qp, kvp, op, qns, kns)

    if "nc" not in _CACHE:
        _CACHE["nc"] = _build()
    nc = _CACHE["nc"]

    halves = []
    for half in range(2):
        halves.append((
            _pack(qp[half * NH:(half + 1) * NH]),
            _pack(kvp[0, half * NKV:(half + 1) * NKV]),
            _pack(kvp[1, half * NKV:(half + 1) * NKV]),
            np.ascontiguousarray(
                op[half * NH:(half + 1) * NH].transpose(1, 0, 2).reshape(128, NH * D)
            ).astype(npbf16),
        ))
    qsc = ((1.0 + qns) / np.sqrt(H)).reshape(128, 1).astype(np.float32)
    ksc = ((1.0 + kns) * np.sqrt(H)).reshape(128, 1).astype(np.float32)
    ts = THETA ** (np.arange(64, dtype=np.float64) * 2.0 / H)
    pidx = np.arange(128)[:, None]
    fidx = np.arange(128)[None, :]
    cm = (fidx >= pidx).astype(npbf16)
    rmat = np.zeros((128, 128), np.float32)
    rmat[(np.arange(128) + 64) % 128, np.arange(128)] = 1.0
    rmat = rmat.astype(npbf16)

    in_maps = []
    for c in range(8):
        b, half = c // 2, c % 2
        wq, wk, wv, wo = halves[half]
        ang = position[b].astype(np.float64)[None, :] / ts[:, None]
        sn = np.sin(ang).astype(np.float32)
        cs = np.cos(ang).astype(np.float32)
        in_maps.append({
            "xt": np.ascontiguousarray(x[b].T).astype(npbf16),
            "wq": wq, "wk": wk, "wv": wv, "wo": wo,
            "cs": np.ascontiguousarray(np.concatenate([cs, cs], 0)).astype(npbf16),
            "sn": np.ascontiguousarray(np.concatenate([-sn, sn], 0)).astype(npbf16),
            "qsc": qsc, "ksc": ksc, "cm": cm, "rmat": rmat,
        })

    if TRACE:
        _install_hook()
    last_err = None
    for _ in range(3):
        try:
            res = bass_utils.run_bass_kernel_spmd(nc, in_maps, list(range(8)), trace=TRACE)
            break
        except Exception as e:  # transient NRT device wedge
            last_err = e
    else:
        raise last_err
    LAST_EXEC_NS = getattr(res, "exec_time_ns", None)

    out = np.empty((B, T, D), np.float32)
    for b in range(B):
        out[b] = (res.results[2 * b]["out"].astype(np.float32)
                  + res.results[2 * b + 1]["out"].astype(np.float32))
    return out


# revision 25
# speedup vs baseline: 1.4415x; 1.0096x over previous
"""MultiHeadAttention (B=4,T=2048,D=2048,NQ=16,NK=8,H=128) on 8 trn2 cores.

Sharding: core c -> batch b=c//2, half=c%2. Each core computes the partial
output for batch b restricted to q-heads [half*8, half*8+8) (kv-heads
[half*4, half*4+4)); host sums the two partials per batch (o_proj
contraction over heads is split across the core pair).

v2: bf16 matmul/vector datapath (PSUM accumulation stays f32), fused
q/k/v projection pass with SBUF-resident q (no DRAM spill), DMA-engine
transposes for V, batched RMSNorm row statistics, f32r broadcast
matmuls, causal-restricted score/exp/PV tiles, software-pipelined
exp/PV attention loop with o_proj fused per 512-row query chunk.
"""
import numpy as np
import concourse.bass as bass
import concourse.tile as tile
from concourse import bacc, mybir
from concourse import bass_utils

B, T, D = 4, 2048, 2048
NQ, NK, H = 16, 8, 128
NH, NKV = 8, 4          # per-core q heads / kv heads
THETA = 10000.0
EPS = 1e-6
TCH = 512               # chunk of T for projections / attention q blocks
NCH = T // TCH
NDK = D // 128
NQC = T // TCH

f32 = mybir.dt.float32
f32r = mybir.dt.float32r
bf16 = mybir.dt.bfloat16
npbf16 = mybir.dt.np(bf16)
AF = mybir.ActivationFunctionType
MUL = mybir.AluOpType.mult

TRACE = False
LAST_EXEC_NS = None
_CACHE = {}


def _install_hook():
    import contextlib, ctypes, sys, types
    if "antenv.axon_hooks" in sys.modules:
        return
    lib = ctypes.CDLL("/opt/axon/libaxon_pjrt.so")
    lib.axon_start_nrt_profile.argtypes = [ctypes.POINTER(ctypes.c_int64), ctypes.c_size_t]
    lib.axon_start_nrt_profile.restype = ctypes.c_int64
    lib.axon_stop_nrt_profile.argtypes = [ctypes.c_char_p]
    lib.axon_stop_nrt_profile.restype = ctypes.c_int64

    @contextlib.contextmanager
    def _hook(output_dir, device_ids):
        import jax
        jax.devices()
        ids = (ctypes.c_int64 * len(device_ids))(*device_ids) if device_ids else None
        rc = lib.axon_start_nrt_profile(ids, len(device_ids) if device_ids else 0)
        if rc != 0:
            raise RuntimeError(f"axon_start_nrt_profile rc={rc}")
        try:
            yield
        finally:
            n = lib.axon_stop_nrt_profile(str(output_dir).encode())
            if n < 0:
                raise RuntimeError(f"axon_stop_nrt_profile rc={n}")

    mod = types.ModuleType("antenv.axon_hooks")
    mod.get_axon_ntff_profile_hook = lambda: _hook
    mod.set_axon_ntff_profile_hook = lambda h: None
    sys.modules["antenv.axon_hooks"] = mod
    bass_utils.upload_artifacts = lambda tmpdir: "local://" + str(tmpdir)


def _build():
    nc = bacc.Bacc("TRN2", target_bir_lowering=False, debug=False, num_devices=8)
    xt_ap = nc.dram_tensor("xt", [D, T], bf16, kind="ExternalInput").ap()
    wq_ap = nc.dram_tensor("wq", [128, NH * NDK * 128], bf16, kind="ExternalInput").ap()
    wk_ap = nc.dram_tensor("wk", [128, NKV * NDK * 128], bf16, kind="ExternalInput").ap()
    wv_ap = nc.dram_tensor("wv", [128, NKV * NDK * 128], bf16, kind="ExternalInput").ap()
    wo_ap = nc.dram_tensor("wo", [128, NH * D], bf16, kind="ExternalInput").ap()
    cs_ap = nc.dram_tensor("cs", [128, T], bf16, kind="ExternalInput").ap()
    sn_ap = nc.dram_tensor("sn", [128, T], bf16, kind="ExternalInput").ap()
    qsc_ap = nc.dram_tensor("qsc", [128, 1], f32, kind="ExternalInput").ap()
    ksc_ap = nc.dram_tensor("ksc", [128, 1], f32, kind="ExternalInput").ap()
    cm_ap = nc.dram_tensor("cm", [128, 128], bf16, kind="ExternalInput").ap()
    rm_ap = nc.dram_tensor("rmat", [128, 128], bf16, kind="ExternalInput").ap()
    out_ap = nc.dram_tensor("out", [T, D], bf16, kind="ExternalOutput").ap()

    with tile.TileContext(nc) as tc:
        with tc.tile_pool(name="mp", bufs=1) as mp, \
             tc.tile_pool(name="pp", bufs=1, space="PSUM") as pp:
            # ---- persistent tiles ----
            cs_t = mp.tile([128, T], bf16)
            nc.sync.dma_start(cs_t[:], cs_ap[:])
            sn_t = mp.tile([128, T], bf16)
            nc.sync.dma_start(sn_t[:], sn_ap[:])
            qsc_t = mp.tile([128, 1], f32)
            nc.sync.dma_start(qsc_t[:], qsc_ap[:])
            ksc_t = mp.tile([128, 1], f32)
            nc.sync.dma_start(ksc_t[:], ksc_ap[:])
            cm_t = mp.tile([128, 128], bf16)
            nc.sync.dma_start(cm_t[:], cm_ap[:])
            rmat_t = mp.tile([128, 128], bf16)
            nc.sync.dma_start(rmat_t[:], rm_ap[:])
            eps_t = mp.tile([1, 1], f32)
            nc.vector.memset(eps_t[:], EPS)
            ones_col_b = mp.tile([128, 1], bf16)
            nc.vector.memset(ones_col_b[:], 1.0)
            ones_row_b = mp.tile([1, 128], bf16)
            nc.vector.memset(ones_row_b[:], 1.0)

            qT = mp.tile([128, NH * T], bf16)     # 32KB/part
            kT = mp.tile([128, NKV * T], bf16)    # 16KB
            vT = mp.tile([128, NKV * T], bf16)    # 16KB

            # PSUM bank for softmax denominator rows (attention phase).
            rows = pp.tile([128, 512], f32, tag="rows")

            def drain_group(pool, accs, sc_t, t0, dsts):
                """accs: psum [128,512] f32 -> RMSNorm*(gain) + RoPE -> dsts bf16."""
                n = len(accs)
                for i in range(n):
                    sq = pool.tile([128, TCH], bf16, tag="sq", bufs=2, name=f"sq{i}")
                    nc.scalar.activation(sq[:], accs[i], AF.Square)
                    row = pp.tile([1, 512], f32, tag=f"a{3 + i % 2}", name=f"row{i}")
                    nc.tensor.matmul(row[:], ones_col_b[:], sq[:], start=True, stop=True)
                    rinv = pool.tile([1, TCH], f32, tag="rinv", bufs=2, name=f"rinv{i}")
                    nc.vector.reciprocal_approx_fast(rinv[:], row[:])
                    rstd = pool.tile([1, TCH], bf16, tag="rstd", bufs=2, name=f"rstd{i}")
                    nc.scalar.activation(rstd[:], rinv[:], AF.Sqrt)
                    bc = pp.tile([128, 512], f32, tag=f"a{3 + (i + 1) % 2}", name=f"bc{i}")
                    nc.tensor.matmul(bc[:], ones_row_b[:], rstd[:],
                                     start=True, stop=True)
                    # qn0 = acc * gain (per-partition); rstd applied after rope
                    # (valid: rstd is per-column, invariant under the half-swap)
                    qn = pool.tile([128, TCH], bf16, tag="qn", bufs=2, name=f"qn{i}")
                    nc.scalar.activation(qn[:], accs[i], AF.Copy, scale=sc_t[:])
                    qsw = pp.tile([128, 512], f32, tag=f"a{5 + i % 2}", name=f"qsw{i}")
                    nc.tensor.matmul(qsw[:], rmat_t[:], qn[:], start=True, stop=True)
                    ta = pool.tile([128, TCH], bf16, tag="ta", bufs=2, name=f"ta{i}")
                    nc.vector.tensor_mul(ta[:], qn[:], cs_t[:, t0:t0 + TCH])
                    tb = pool.tile([128, TCH], bf16, tag="tb", bufs=2, name=f"tb{i}")
                    nc.vector.tensor_mul(tb[:], qsw[:], sn_t[:, t0:t0 + TCH])
                    rs = pool.tile([128, TCH], bf16, tag="rs", bufs=2, name=f"rs{i}")
                    nc.vector.tensor_add(rs[:], ta[:], tb[:])
                    nc.vector.tensor_mul(dsts[i], rs[:], bc[:])

            # ---- phase 1: q/k/v projection, norm+rope, all SBUF-resident ----
            with tc.tile_pool(name="pj", bufs=1) as pj:
                wq_t = pj.tile([128, NH * NDK * 128], bf16)
                for j in range(4):
                    s = NH * NDK * 128 // 4
                    nc.sync.dma_start(wq_t[:, j * s:(j + 1) * s], wq_ap[:, j * s:(j + 1) * s])
                wk_t = pj.tile([128, NKV * NDK * 128], bf16)
                for j in range(2):
                    s = NKV * NDK * 128 // 2
                    nc.sync.dma_start(wk_t[:, j * s:(j + 1) * s], wk_ap[:, j * s:(j + 1) * s])
                wv_t = pj.tile([128, NKV * NDK * 128], bf16)
                for j in range(2):
                    s = NKV * NDK * 128 // 2
                    nc.sync.dma_start(wv_t[:, j * s:(j + 1) * s], wv_ap[:, j * s:(j + 1) * s])

                acc_roll = [0]

                def chain(w_t, head, xh0, xh1):
                    """Sequential 16-dk matmul chain into one rolling psum bank."""
                    acc = pp.tile([128, 512], f32, tag=f"a{acc_roll[0] % 3}", name="acc")
                    acc_roll[0] += 1
                    for dk in range(NDK):
                        xh = xh0 if dk < 8 else xh1
                        nc.tensor.matmul(
                            acc[:],
                            w_t[:, (head * NDK + dk) * 128:(head * NDK + dk + 1) * 128],
                            xh[:, (dk % 8) * TCH:(dk % 8 + 1) * TCH],
                            start=(dk == 0), stop=(dk == NDK - 1))
                    return acc

                xt_r = xt_ap.rearrange("(a p) t -> p a t", p=128)

                def load_xh(ch):
                    t0 = ch * TCH
                    xh0 = pj.tile([128, 8 * TCH], bf16, tag="xh0", bufs=2, name="xh0")
                    nc.sync.dma_start(
                        xh0[:].rearrange("p (a t) -> p a t", a=8),
                        xt_r[:, 0:8, t0:t0 + TCH])
                    xh1 = pj.tile([128, 8 * TCH], bf16, tag="xh1", bufs=2, name="xh1")
                    nc.sync.dma_start(
                        xh1[:].rearrange("p (a t) -> p a t", a=8),
                        xt_r[:, 8:16, t0:t0 + TCH])
                    return xh0, xh1

                xhs = load_xh(0)
                for ch in range(NCH):
                    t0 = ch * TCH
                    xh0, xh1 = xhs
                    # prefetch next chunk's x before this chunk's drains/V
                    # DMAs enter the in-order SP queue
                    if ch + 1 < NCH:
                        xhs = load_xh(ch + 1)
                    # q heads, two groups of 4
                    for g in range(2):
                        accs = [chain(wq_t, g * 4 + i, xh0, xh1) for i in range(4)]
                        drain_group(pj, [a[:] for a in accs], qsc_t, t0,
                                    [qT[:, (g * 4 + i) * T + t0:(g * 4 + i) * T + t0 + TCH]
                                     for i in range(4)])
                    # k heads
                    accs = [chain(wk_t, i, xh0, xh1) for i in range(NKV)]
                    drain_group(pj, [a[:] for a in accs], ksc_t, t0,
                                [kT[:, kv * T + t0:kv * T + t0 + TCH] for kv in range(NKV)])
                    # v heads: computed directly in [t, H] orientation
                    # (lhsT = x block, rhs = w column), so no transpose needed
                    for kv in range(NKV):
                        vps = pp.tile([128, 512], f32, tag=f"a{acc_roll[0] % 3}", name="vps")
                        acc_roll[0] += 1
                        for tb_ in range(4):
                            for dk in range(NDK):
                                xh = xh0 if dk < 8 else xh1
                                nc.tensor.matmul(
                                    vps[:, tb_ * 128:(tb_ + 1) * 128],
                                    xh[:, (dk % 8) * TCH + tb_ * 128:(dk % 8) * TCH + (tb_ + 1) * 128],
                                    wv_t[:, (kv * NDK + dk) * 128:(kv * NDK + dk + 1) * 128],
                                    start=(dk == 0), stop=(dk == NDK - 1),
                                    skip_group_check=True)
                        nc.vector.tensor_copy(vT[:, kv * T + t0:kv * T + t0 + TCH], vps[:])

            # ---- phase 2: attention + fused o_proj ----
            with tc.tile_pool(name="op", bufs=1) as op:
                wo_t = op.tile([128, NH * D], bf16)
                for j in range(4):
                    s = NH * D // 4
                    nc.sync.dma_start(wo_t[:, j * s:(j + 1) * s], wo_ap[:, j * s:(j + 1) * s])

                for qi in range(NQC):
                    q0 = qi * TCH
                    attn_sb = op.tile([128, NH * TCH], bf16, tag="attn", bufs=2, name="attn")
                    nkj = 4 * qi + 4
                    for h in range(NH):
                        kv = h // 2
                        o_ps = pp.tile([128, 512], f32, tag=f"a{5 + h % 2}", name="ops")
                        acc_sb = op.tile([128, TCH], bf16, tag="accsb", bufs=3, name="accsb")

                        def emit_s(kj):
                            m = kj - 4 * qi
                            lo = 128 * m if m > 0 else 0
                            s_ps = pp.tile([128, 512], f32, tag=f"a{kj % 5}", name="sps")
                            nc.tensor.matmul(
                                s_ps[:, lo:512],
                                kT[:, kv * T + kj * 128:kv * T + (kj + 1) * 128],
                                qT[:, h * T + q0 + lo:h * T + q0 + TCH],
                                start=True, stop=True)
                            pt = op.tile([128, TCH], bf16, tag="pt", bufs=4, name="pt")
                            return s_ps, pt, lo, m, kj

                        def emit_drain(s_ps, pt, lo, m, kj):
                            nc.scalar.activation(pt[:, lo:512], s_ps[:, lo:512], AF.Exp)
                            if m >= 0:
                                nc.vector.tensor_mul(pt[:, lo:lo + 128],
                                                     pt[:, lo:lo + 128], cm_t[:])
                            if kj == 0:
                                nc.vector.tensor_copy(acc_sb[:], pt[:])
                            else:
                                nc.vector.tensor_add(acc_sb[:, lo:512],
                                                     acc_sb[:, lo:512], pt[:, lo:512])
                            nc.tensor.matmul(
                                o_ps[:, lo:512],
                                vT[:, kv * T + kj * 128:kv * T + (kj + 1) * 128],
                                pt[:, lo:512],
                                start=(kj == 0), stop=(kj == nkj - 1),
                                skip_group_check=True)

                        prev = None
                        for kj in range(nkj):
                            cur = emit_s(kj)
                            if prev is not None:
                                emit_drain(*prev)
                            prev = cur
                        emit_drain(*prev)

                        # softmax denominator for this (h, qi).
                        # NB: reciprocal_approx_fast corrupts results when its
                        # input sits at a nonzero base partition -> keep row 0.
                        nc.tensor.matmul(rows[0:1, :], ones_col_b[:],
                                         acc_sb[:], start=True, stop=True)
                        rden = op.tile([1, TCH], f32, tag="rden", bufs=3, name="rden")
                        nc.vector.reciprocal_approx_fast(rden[:], rows[0:1, :])
                        rbc = op.tile([128, TCH], f32, tag="rbc", bufs=3, name="rbc")
                        nc.gpsimd.partition_broadcast(rbc[:], rden[:])
                        nc.vector.tensor_mul(
                            attn_sb[:, h * TCH:(h + 1) * TCH], o_ps[:], rbc[:])

                    # fused o_proj for this 512-row query chunk
                    out_r = out_ap.rearrange("(a p) d -> p a d", p=128)
                    for dc in range(4):
                        stg4 = op.tile([128, 4 * 512], bf16, tag="ostg", bufs=2, name="ostg")
                        for ti in range(4):
                            ops2 = pp.tile([128, 512], f32,
                                           tag=f"a{5 + (dc * 4 + ti) % 2}", name="ops2")
                            for h in range(NH):
                                nc.tensor.matmul(
                                    ops2[:],
                                    attn_sb[:, h * TCH + ti * 128:h * TCH + (ti + 1) * 128],
                                    wo_t[:, h * D + dc * 512:h * D + (dc + 1) * 512],
                                    start=(h == 0), stop=(h == NH - 1))
                            if (dc * 4 + ti) % 2 == 0:
                                nc.vector.tensor_copy(stg4[:, ti * 512:(ti + 1) * 512], ops2[:])
                            else:
                                nc.scalar.activation(stg4[:, ti * 512:(ti + 1) * 512],
                                                     ops2[:], AF.Copy)
                        nc.sync.dma_start(
                            out_r[:, qi * 4:qi * 4 + 4, dc * 512:(dc + 1) * 512],
                            stg4[:].rearrange("p (a d) -> p a d", a=4))

    nc.compile()
    return nc


def _pack(w):
    """(nh, D, H) -> (128, nh*NDK*128): col block (h*NDK+dk)*128 = w[h, dk*128:+128, :]."""
    nh = w.shape[0]
    a = w.reshape(nh, NDK, 128, H).transpose(2, 0, 1, 3)
    return np.ascontiguousarray(a.reshape(128, nh * NDK * H)).astype(npbf16)


def _numpy_ref(x, mask, position, qp, kvp, op, qns, kns):
    def rms(v, s):
        var = (v * v).mean(-1, keepdims=True)
        return v / np.sqrt(var + EPS) * (1.0 + s)

    def rope(v, pos):
        ts = THETA ** (np.arange(64, dtype=np.float32) * 2.0 / H)
        ang = pos.astype(np.float32)[:, :, None, None] / ts
        sn, cs = np.sin(ang), np.cos(ang)
        x1, x2 = v[..., :64], v[..., 64:]
        return np.concatenate([x1 * cs - x2 * sn, x2 * cs + x1 * sn], -1)

    q = np.einsum('BTD,NDH->BTNH', x, qp)
    k = np.einsum('BTD,KDH->BTKH', x, kvp[0])
    v = np.einsum('BTD,KDH->BTKH', x, kvp[1])
    q = rope(rms(q, qns), position) * (H ** -0.5)
    k = rope(rms(k, kns), position)
    q = q.transpose(0, 2, 1, 3)
    k = np.repeat(k.transpose(0, 2, 1, 3), NQ // NK, 1)
    v = np.repeat(v.transpose(0, 2, 1, 3), NQ // NK, 1)
    s = np.einsum('BHtD,BHTD->BHtT', q, k) / np.sqrt(np.float32(H))
    s = np.where(mask[:, None], s, np.float32(-2.3819763e+38))
    s = s - s.max(-1, keepdims=True)
    w = np.exp(s)
    w /= w.sum(-1, keepdims=True)
    o = np.einsum('BHtT,BHTD->BHtD', w, v)
    return np.einsum('BNTH,NHD->BTD', o, op).astype(np.float32)


def kernel(**inputs):
    global LAST_EXEC_NS
    x = np.asarray(inputs["x"], np.float32)
    mask = np.asarray(inputs["mask"])
    position = np.asarray(inputs["position"])
    qp = np.asarray(inputs["q_proj"], np.float32)
    kvp = np.asarray(inputs["kv_proj"], np.float32)
    op = np.asarray(inputs["o_proj"], np.float32)
    qns = np.asarray(inputs["q_norm_scale"], np.float32)
    kns = np.asarray(inputs["k_norm_scale"], np.float32)

    tril = np.tril(np.ones((T, T), bool))
    if mask.shape != (B, T, T) or not all(np.array_equal(mask[b], tril) for b in range(B)):
        return _numpy_ref(x, mask, position, qp, kvp, op, qns, kns)

    if "nc" not in _CACHE:
        _CACHE["nc"] = _build()
    nc = _CACHE["nc"]

    halves = []
    for half in range(2):
        halves.append((
            _pack(qp[half * NH:(half + 1) * NH]),
            _pack(kvp[0, half * NKV:(half + 1) * NKV]),
            _pack(kvp[1, half * NKV:(half + 1) * NKV]),
            np.ascontiguousarray(
                op[half * NH:(half + 1) * NH].transpose(1, 0, 2).reshape(128, NH * D)
            ).astype(npbf16),
        ))
    qsc = ((1.0 + qns) / np.sqrt(H)).reshape(128, 1).astype(np.float32)
    ksc = ((1.0 + kns) * np.sqrt(H)).reshape(128, 1).astype(np.float32)
    ts = THETA ** (np.arange(64, dtype=np.float64) * 2.0 / H)
    pidx = np.arange(128)[:, None]
    fidx = np.arange(128)[None, :]
    cm = (fidx >= pidx).astype(npbf16)
    rmat = np.zeros((128, 128), np.float32)
    rmat[(np.arange(128) + 64) % 128, np.arange(128)] = 1.0
    rmat = rmat.astype(npbf16)

    in_maps = []
    for c in range(8):
        b, half = c // 2, c % 2
        wq, wk, wv, wo = halves[half]
        ang = position[b].astype(np.float64)[None, :] / ts[:, None]
        sn = np.sin(ang).astype(np.float32)
        cs = np.cos(ang).astype(np.float32)
        in_maps.append({
            "xt": np.ascontiguousarray(x[b].T).astype(npbf16),
            "wq": wq, "wk": wk, "wv": wv, "wo": wo,
            "cs": np.ascontiguousarray(np.concatenate([cs, cs], 0)).astype(npbf16),
            "sn": np.ascontiguousarray(np.concatenate([-sn, sn], 0)).astype(npbf16),
            "qsc": qsc, "ksc": ksc, "cm": cm, "rmat": rmat,
        })

    if TRACE:
        _install_hook()
    last_err = None
    for _ in range(3):
        try:
            res = bass_utils.run_bass_kernel_spmd(nc, in_maps, list(range(8)), trace=TRACE)
            break
        except Exception as e:  # transient NRT device wedge
            last_err = e
    else:
        raise last_err
    LAST_EXEC_NS = getattr(res, "exec_time_ns", None)

    out = np.empty((B, T, D), np.float32)
    for b in range(B):
        out[b] = (res.results[2 * b]["out"].astype(np.float32)
                  + res.results[2 * b + 1]["out"].astype(np.float32))
    return out


# revision 26
# speedup vs baseline: 1.4553x; 1.0096x over previous
"""MultiHeadAttention (B=4,T=2048,D=2048,NQ=16,NK=8,H=128) on 8 trn2 cores.

Sharding: core c -> batch b=c//2, half=c%2. Each core computes the partial
output for batch b restricted to q-heads [half*8, half*8+8) (kv-heads
[half*4, half*4+4)); host sums the two partials per batch (o_proj
contraction over heads is split across the core pair).

v2: bf16 matmul/vector datapath (PSUM accumulation stays f32), fused
q/k/v projection pass with SBUF-resident q (no DRAM spill), DMA-engine
transposes for V, batched RMSNorm row statistics, f32r broadcast
matmuls, causal-restricted score/exp/PV tiles, software-pipelined
exp/PV attention loop with o_proj fused per 512-row query chunk.
"""
import numpy as np
import concourse.bass as bass
import concourse.tile as tile
from concourse import bacc, mybir
from concourse import bass_utils

B, T, D = 4, 2048, 2048
NQ, NK, H = 16, 8, 128
NH, NKV = 8, 4          # per-core q heads / kv heads
THETA = 10000.0
EPS = 1e-6
TCH = 512               # chunk of T for projections / attention q blocks
NCH = T // TCH
NDK = D // 128
NQC = T // TCH

f32 = mybir.dt.float32
f32r = mybir.dt.float32r
bf16 = mybir.dt.bfloat16
npbf16 = mybir.dt.np(bf16)
AF = mybir.ActivationFunctionType
MUL = mybir.AluOpType.mult

TRACE = False
LAST_EXEC_NS = None
_CACHE = {}


def _install_hook():
    import contextlib, ctypes, sys, types
    if "antenv.axon_hooks" in sys.modules:
        return
    lib = ctypes.CDLL("/opt/axon/libaxon_pjrt.so")
    lib.axon_start_nrt_profile.argtypes = [ctypes.POINTER(ctypes.c_int64), ctypes.c_size_t]
    lib.axon_start_nrt_profile.restype = ctypes.c_int64
    lib.axon_stop_nrt_profile.argtypes = [ctypes.c_char_p]
    lib.axon_stop_nrt_profile.restype = ctypes.c_int64

    @contextlib.contextmanager
    def _hook(output_dir, device_ids):
        import jax
        jax.devices()
        ids = (ctypes.c_int64 * len(device_ids))(*device_ids) if device_ids else None
        rc = lib.axon_start_nrt_profile(ids, len(device_ids) if device_ids else 0)
        if rc != 0:
            raise RuntimeError(f"axon_start_nrt_profile rc={rc}")
        try:
            yield
        finally:
            n = lib.axon_stop_nrt_profile(str(output_dir).encode())
            if n < 0:
                raise RuntimeError(f"axon_stop_nrt_profile rc={n}")

    mod = types.ModuleType("antenv.axon_hooks")
    mod.get_axon_ntff_profile_hook = lambda: _hook
    mod.set_axon_ntff_profile_hook = lambda h: None
    sys.modules["antenv.axon_hooks"] = mod
    bass_utils.upload_artifacts = lambda tmpdir: "local://" + str(tmpdir)


def _build():
    nc = bacc.Bacc("TRN2", target_bir_lowering=False, debug=False, num_devices=8)
    xt_ap = nc.dram_tensor("xt", [D, T], bf16, kind="ExternalInput").ap()
    wq_ap = nc.dram_tensor("wq", [128, NH * NDK * 128], bf16, kind="ExternalInput").ap()
    wk_ap = nc.dram_tensor("wk", [128, NKV * NDK * 128], bf16, kind="ExternalInput").ap()
    wv_ap = nc.dram_tensor("wv", [128, NKV * NDK * 128], bf16, kind="ExternalInput").ap()
    wo_ap = nc.dram_tensor("wo", [128, NH * D], bf16, kind="ExternalInput").ap()
    cs_ap = nc.dram_tensor("cs", [128, T], bf16, kind="ExternalInput").ap()
    sn_ap = nc.dram_tensor("sn", [128, T], bf16, kind="ExternalInput").ap()
    qsc_ap = nc.dram_tensor("qsc", [128, 1], f32, kind="ExternalInput").ap()
    ksc_ap = nc.dram_tensor("ksc", [128, 1], f32, kind="ExternalInput").ap()
    cm_ap = nc.dram_tensor("cm", [128, 128], bf16, kind="ExternalInput").ap()
    rm_ap = nc.dram_tensor("rmat", [128, 128], bf16, kind="ExternalInput").ap()
    out_ap = nc.dram_tensor("out", [T, D], bf16, kind="ExternalOutput").ap()

    with tile.TileContext(nc) as tc:
        with tc.tile_pool(name="mp", bufs=1) as mp, \
             tc.tile_pool(name="pp", bufs=1, space="PSUM") as pp:
            # ---- persistent tiles ----
            cs_t = mp.tile([128, T], bf16)
            nc.sync.dma_start(cs_t[:], cs_ap[:])
            sn_t = mp.tile([128, T], bf16)
            nc.sync.dma_start(sn_t[:], sn_ap[:])
            qsc_t = mp.tile([128, 1], f32)
            nc.sync.dma_start(qsc_t[:], qsc_ap[:])
            ksc_t = mp.tile([128, 1], f32)
            nc.sync.dma_start(ksc_t[:], ksc_ap[:])
            cm_t = mp.tile([128, 128], bf16)
            nc.sync.dma_start(cm_t[:], cm_ap[:])
            rmat_t = mp.tile([128, 128], bf16)
            nc.sync.dma_start(rmat_t[:], rm_ap[:])
            eps_t = mp.tile([1, 1], f32)
            nc.vector.memset(eps_t[:], EPS)
            ones_col_b = mp.tile([128, 1], bf16)
            nc.vector.memset(ones_col_b[:], 1.0)
            ones_row_b = mp.tile([1, 128], bf16)
            nc.vector.memset(ones_row_b[:], 1.0)

            qT = mp.tile([128, NH * T], bf16)     # 32KB/part
            kT = mp.tile([128, NKV * T], bf16)    # 16KB
            vT = mp.tile([128, NKV * T], bf16)    # 16KB

            # PSUM bank for softmax denominator rows (attention phase).
            rows = pp.tile([128, 512], f32, tag="rows")

            def drain_group(pool, accs, sc_t, t0, dsts):
                """accs: psum [128,512] f32 -> RMSNorm*(gain) + RoPE -> dsts bf16."""
                n = len(accs)
                for i in range(n):
                    sq = pool.tile([128, TCH], bf16, tag="sq", bufs=2, name=f"sq{i}")
                    nc.scalar.activation(sq[:], accs[i], AF.Square)
                    row = pp.tile([1, 512], f32, tag=f"a{3 + i % 2}", name=f"row{i}")
                    nc.tensor.matmul(row[:], ones_col_b[:], sq[:], start=True, stop=True)
                    rinv = pool.tile([1, TCH], f32, tag="rinv", bufs=2, name=f"rinv{i}")
                    nc.vector.reciprocal_approx_fast(rinv[:], row[:])
                    rstd = pool.tile([1, TCH], bf16, tag="rstd", bufs=2, name=f"rstd{i}")
                    nc.scalar.activation(rstd[:], rinv[:], AF.Sqrt)
                    bc = pp.tile([128, 512], f32, tag=f"a{3 + (i + 1) % 2}", name=f"bc{i}")
                    nc.tensor.matmul(bc[:], ones_row_b[:], rstd[:],
                                     start=True, stop=True)
                    # qn0 = acc * gain (per-partition); rstd applied after rope
                    # (valid: rstd is per-column, invariant under the half-swap)
                    qn = pool.tile([128, TCH], bf16, tag="qn", bufs=2, name=f"qn{i}")
                    nc.scalar.activation(qn[:], accs[i], AF.Copy, scale=sc_t[:])
                    qsw = pp.tile([128, 512], f32, tag=f"a{5 + i % 2}", name=f"qsw{i}")
                    nc.tensor.matmul(qsw[:], rmat_t[:], qn[:], start=True, stop=True)
                    ta = pool.tile([128, TCH], bf16, tag="ta", bufs=2, name=f"ta{i}")
                    nc.vector.tensor_mul(ta[:], qn[:], cs_t[:, t0:t0 + TCH])
                    tb = pool.tile([128, TCH], bf16, tag="tb", bufs=2, name=f"tb{i}")
                    nc.vector.tensor_mul(tb[:], qsw[:], sn_t[:, t0:t0 + TCH])
                    rs = pool.tile([128, TCH], bf16, tag="rs", bufs=2, name=f"rs{i}")
                    nc.vector.tensor_add(rs[:], ta[:], tb[:])
                    nc.vector.tensor_mul(dsts[i], rs[:], bc[:])

            # ---- phase 1: q/k/v projection, norm+rope, all SBUF-resident ----
            with tc.tile_pool(name="pj", bufs=1) as pj:
                wq_t = pj.tile([128, NH * NDK * 128], bf16)
                for j in range(4):
                    s = NH * NDK * 128 // 4
                    nc.sync.dma_start(wq_t[:, j * s:(j + 1) * s], wq_ap[:, j * s:(j + 1) * s])
                wk_t = pj.tile([128, NKV * NDK * 128], bf16)
                for j in range(2):
                    s = NKV * NDK * 128 // 2
                    nc.sync.dma_start(wk_t[:, j * s:(j + 1) * s], wk_ap[:, j * s:(j + 1) * s])
                wv_t = pj.tile([128, NKV * NDK * 128], bf16)
                for j in range(2):
                    s = NKV * NDK * 128 // 2
                    nc.sync.dma_start(wv_t[:, j * s:(j + 1) * s], wv_ap[:, j * s:(j + 1) * s])

                acc_roll = [0]

                def chain(w_t, head, xh0, xh1):
                    """Sequential 16-dk matmul chain into one rolling psum bank."""
                    acc = pp.tile([128, 512], f32, tag=f"a{acc_roll[0] % 3}", name="acc")
                    acc_roll[0] += 1
                    for dk in range(NDK):
                        xh = xh0 if dk < 8 else xh1
                        nc.tensor.matmul(
                            acc[:],
                            w_t[:, (head * NDK + dk) * 128:(head * NDK + dk + 1) * 128],
                            xh[:, (dk % 8) * TCH:(dk % 8 + 1) * TCH],
                            start=(dk == 0), stop=(dk == NDK - 1))
                    return acc

                xt_r = xt_ap.rearrange("(a p) t -> p a t", p=128)

                def load_xh(ch):
                    t0 = ch * TCH
                    xh0 = pj.tile([128, 8 * TCH], bf16, tag="xh0", bufs=2, name="xh0")
                    nc.sync.dma_start(
                        xh0[:].rearrange("p (a t) -> p a t", a=8),
                        xt_r[:, 0:8, t0:t0 + TCH])
                    xh1 = pj.tile([128, 8 * TCH], bf16, tag="xh1", bufs=2, name="xh1")
                    nc.sync.dma_start(
                        xh1[:].rearrange("p (a t) -> p a t", a=8),
                        xt_r[:, 8:16, t0:t0 + TCH])
                    return xh0, xh1

                xhs = load_xh(0)
                for ch in range(NCH):
                    t0 = ch * TCH
                    xh0, xh1 = xhs
                    # prefetch next chunk's x before this chunk's drains/V
                    # DMAs enter the in-order SP queue
                    if ch + 1 < NCH:
                        xhs = load_xh(ch + 1)
                    # q heads, two groups of 4
                    for g in range(2):
                        accs = [chain(wq_t, g * 4 + i, xh0, xh1) for i in range(4)]
                        drain_group(pj, [a[:] for a in accs], qsc_t, t0,
                                    [qT[:, (g * 4 + i) * T + t0:(g * 4 + i) * T + t0 + TCH]
                                     for i in range(4)])
                    # k heads
                    accs = [chain(wk_t, i, xh0, xh1) for i in range(NKV)]
                    drain_group(pj, [a[:] for a in accs], ksc_t, t0,
                                [kT[:, kv * T + t0:kv * T + t0 + TCH] for kv in range(NKV)])
                    # v heads: computed directly in [t, H] orientation
                    # (lhsT = x block, rhs = w column), so no transpose needed
                    for kv in range(NKV):
                        vps = pp.tile([128, 512], f32, tag=f"a{acc_roll[0] % 3}", name="vps")
                        acc_roll[0] += 1
                        for tb_ in range(4):
                            for dk in range(NDK):
                                xh = xh0 if dk < 8 else xh1
                                nc.tensor.matmul(
                                    vps[:, tb_ * 128:(tb_ + 1) * 128],
                                    xh[:, (dk % 8) * TCH + tb_ * 128:(dk % 8) * TCH + (tb_ + 1) * 128],
                                    wv_t[:, (kv * NDK + dk) * 128:(kv * NDK + dk + 1) * 128],
                                    start=(dk == 0), stop=(dk == NDK - 1),
                                    skip_group_check=True)
                        nc.vector.tensor_copy(vT[:, kv * T + t0:kv * T + t0 + TCH], vps[:])

            # ---- phase 2: attention + fused o_proj ----
            with tc.tile_pool(name="op", bufs=1) as op:
                wo_t = op.tile([128, NH * D], bf16)
                for j in range(4):
                    s = NH * D // 4
                    nc.sync.dma_start(wo_t[:, j * s:(j + 1) * s], wo_ap[:, j * s:(j + 1) * s])

                for qi in range(NQC):
                    q0 = qi * TCH
                    attn_sb = op.tile([128, NH * TCH], bf16, tag="attn", bufs=2, name="attn")
                    nkj = 4 * qi + 4
                    for h in range(NH):
                        kv = h // 2
                        o_ps = pp.tile([128, 512], f32, tag=f"a{5 + h % 2}", name="ops")
                        acc_sb = op.tile([128, TCH], bf16, tag="accsb", bufs=3, name="accsb")

                        def emit_s(kj):
                            m = kj - 4 * qi
                            lo = 128 * m if m > 0 else 0
                            s_ps = pp.tile([128, 512], f32, tag=f"a{kj % 5}", name="sps")
                            nc.tensor.matmul(
                                s_ps[:, lo:512],
                                kT[:, kv * T + kj * 128:kv * T + (kj + 1) * 128],
                                qT[:, h * T + q0 + lo:h * T + q0 + TCH],
                                start=True, stop=True)
                            pt = op.tile([128, TCH], bf16, tag="pt", bufs=6, name="pt")
                            return s_ps, pt, lo, m, kj

                        def emit_drain(s_ps, pt, lo, m, kj):
                            nc.scalar.activation(pt[:, lo:512], s_ps[:, lo:512], AF.Exp)
                            if m >= 0:
                                nc.vector.tensor_mul(pt[:, lo:lo + 128],
                                                     pt[:, lo:lo + 128], cm_t[:])
                            if kj == 0:
                                nc.vector.tensor_copy(acc_sb[:], pt[:])
                            else:
                                nc.vector.tensor_add(acc_sb[:, lo:512],
                                                     acc_sb[:, lo:512], pt[:, lo:512])
                            nc.tensor.matmul(
                                o_ps[:, lo:512],
                                vT[:, kv * T + kj * 128:kv * T + (kj + 1) * 128],
                                pt[:, lo:512],
                                start=(kj == 0), stop=(kj == nkj - 1),
                                skip_group_check=True)

                        prev = None
                        for kj in range(nkj):
                            cur = emit_s(kj)
                            if prev is not None:
                                emit_drain(*prev)
                            prev = cur
                        emit_drain(*prev)

                        # softmax denominator for this (h, qi).
                        # NB: reciprocal_approx_fast corrupts results when its
                        # input sits at a nonzero base partition -> keep row 0.
                        nc.tensor.matmul(rows[0:1, :], ones_col_b[:],
                                         acc_sb[:], start=True, stop=True)
                        rden = op.tile([1, TCH], f32, tag="rden", bufs=3, name="rden")
                        nc.vector.reciprocal_approx_fast(rden[:], rows[0:1, :])
                        rbc = op.tile([128, TCH], f32, tag="rbc", bufs=3, name="rbc")
                        nc.gpsimd.partition_broadcast(rbc[:], rden[:])
                        nc.vector.tensor_mul(
                            attn_sb[:, h * TCH:(h + 1) * TCH], o_ps[:], rbc[:])

                    # fused o_proj for this 512-row query chunk
                    out_r = out_ap.rearrange("(a p) d -> p a d", p=128)
                    for dc in range(4):
                        stg4 = op.tile([128, 4 * 512], bf16, tag="ostg", bufs=2, name="ostg")
                        for ti in range(4):
                            ops2 = pp.tile([128, 512], f32,
                                           tag=f"a{4 + (dc * 4 + ti) % 3}", name="ops2")
                            for h in range(NH):
                                nc.tensor.matmul(
                                    ops2[:],
                                    attn_sb[:, h * TCH + ti * 128:h * TCH + (ti + 1) * 128],
                                    wo_t[:, h * D + dc * 512:h * D + (dc + 1) * 512],
                                    start=(h == 0), stop=(h == NH - 1))
                            if (dc * 4 + ti) % 2 == 0:
                                nc.vector.tensor_copy(stg4[:, ti * 512:(ti + 1) * 512], ops2[:])
                            else:
                                nc.scalar.activation(stg4[:, ti * 512:(ti + 1) * 512],
                                                     ops2[:], AF.Copy)
                        nc.sync.dma_start(
                            out_r[:, qi * 4:qi * 4 + 4, dc * 512:(dc + 1) * 512],
                            stg4[:].rearrange("p (a d) -> p a d", a=4))

    nc.compile()
    return nc


def _pack(w):
    """(nh, D, H) -> (128, nh*NDK*128): col block (h*NDK+dk)*128 = w[h, dk*128:+128, :]."""
    nh = w.shape[0]
    a = w.reshape(nh, NDK, 128, H).transpose(2, 0, 1, 3)
    return np.ascontiguousarray(a.reshape(128, nh * NDK * H)).astype(npbf16)


def _numpy_ref(x, mask, position, qp, kvp, op, qns, kns):
    def rms(v, s):
        var = (v * v).mean(-1, keepdims=True)
        return v / np.sqrt(var + EPS) * (1.0 + s)

    def rope(v, pos):
        ts = THETA ** (np.arange(64, dtype=np.float32) * 2.0 / H)
        ang = pos.astype(np.float32)[:, :, None, None] / ts
        sn, cs = np.sin(ang), np.cos(ang)
        x1, x2 = v[..., :64], v[..., 64:]
        return np.concatenate([x1 * cs - x2 * sn, x2 * cs + x1 * sn], -1)

    q = np.einsum('BTD,NDH->BTNH', x, qp)
    k = np.einsum('BTD,KDH->BTKH', x, kvp[0])
    v = np.einsum('BTD,KDH->BTKH', x, kvp[1])
    q = rope(rms(q, qns), position) * (H ** -0.5)
    k = rope(rms(k, kns), position)
    q = q.transpose(0, 2, 1, 3)
    k = np.repeat(k.transpose(0, 2, 1, 3), NQ // NK, 1)
    v = np.repeat(v.transpose(0, 2, 1, 3), NQ // NK, 1)
    s = np.einsum('BHtD,BHTD->BHtT', q, k) / np.sqrt(np.float32(H))
    s = np.where(mask[:, None], s, np.float32(-2.3819763e+38))
    s = s - s.max(-1, keepdims=True)
    w = np.exp(s)
    w /= w.sum(-1, keepdims=True)
    o = np.einsum('BHtT,BHTD->BHtD', w, v)
    return np.einsum('BNTH,NHD->BTD', o, op).astype(np.float32)


def kernel(**inputs):
    global LAST_EXEC_NS
    x = np.asarray(inputs["x"], np.float32)
    mask = np.asarray(inputs["mask"])
    position = np.asarray(inputs["position"])
    qp = np.asarray(inputs["q_proj"], np.float32)
    kvp = np.asarray(inputs["kv_proj"], np.float32)
    op = np.asarray(inputs["o_proj"], np.float32)
    qns = np.asarray(inputs["q_norm_scale"], np.float32)
    kns = np.asarray(inputs["k_norm_scale"], np.float32)

    tril = np.tril(np.ones((T, T), bool))
    if mask.shape != (B, T, T) or not all(np.array_equal(mask[b], tril) for b in range(B)):
        return _numpy_ref(x, mask, position, qp, kvp, op, qns, kns)

    if "nc" not in _CACHE:
        _CACHE["nc"] = _build()
    nc = _CACHE["nc"]

    halves = []
    for half in range(2):
        halves.append((
            _pack(qp[half * NH:(half + 1) * NH]),
            _pack(kvp[0, half * NKV:(half + 1) * NKV]),
            _pack(kvp[1, half * NKV:(half + 1) * NKV]),
            np.ascontiguousarray(
                op[half * NH:(half + 1) * NH].transpose(1, 0, 2).reshape(128, NH * D)
            ).astype(npbf16),
        ))
    qsc = ((1.0 + qns) / np.sqrt(H)).reshape(128, 1).astype(np.float32)
    ksc = ((1.0 + kns) * np.sqrt(H)).reshape(128, 1).astype(np.float32)
    ts = THETA ** (np.arange(64, dtype=np.float64) * 2.0 / H)
    pidx = np.arange(128)[:, None]
    fidx = np.arange(128)[None, :]
    cm = (fidx >= pidx).astype(npbf16)
    rmat = np.zeros((128, 128), np.float32)
    rmat[(np.arange(128) + 64) % 128, np.arange(128)] = 1.0
    rmat = rmat.astype(npbf16)

    in_maps = []
    for c in range(8):
        b, half = c // 2, c % 2
        wq, wk, wv, wo = halves[half]
        ang = position[b].astype(np.float64)[None, :] / ts[:, None]
        sn = np.sin(ang).astype(np.float32)
        cs = np.cos(ang).astype(np.float32)
        in_maps.append({
            "xt": np.ascontiguousarray(x[b].T).astype(npbf16),
            "wq": wq, "wk": wk, "wv": wv, "wo": wo,
            "cs": np.ascontiguousarray(np.concatenate([cs, cs], 0)).astype(npbf16),
            "sn": np.ascontiguousarray(np.concatenate([-sn, sn], 0)).astype(npbf16),
            "qsc": qsc, "ksc": ksc, "cm": cm, "rmat": rmat,
        })

    if TRACE:
        _install_hook()
    last_err = None
    for _ in range(3):
        try:
            res = bass_utils.run_bass_kernel_spmd(nc, in_maps, list(range(8)), trace=TRACE)
            break
        except Exception as e:  # transient NRT device wedge
            last_err = e
    else:
        raise last_err
    LAST_EXEC_NS = getattr(res, "exec_time_ns", None)

    out = np.empty((B, T, D), np.float32)
    for b in range(B):
        out[b] = (res.results[2 * b]["out"].astype(np.float32)
                  + res.results[2 * b + 1]["out"].astype(np.float32))
    return out
